# revision 6
# baseline (speedup 1.0000x reference)
"""Trainium2 Bass kernel for a dense transformer block (nn_Block_120259084502).

fp8 DoubleRow variant: attention matmuls (Q/K/V projections, attn@V, output
proj) run in fp8-e4m3 with DoubleRow perf mode (2 contraction elements per
cycle -> half the matmul streams). Scores (K=64/head, no DR gain) and the
MLP (fp8 fails the error budget there) stay bf16. Softmax runs shifted
(exp(S-2)) so probabilities fit fp8-e4m3's +/-240 range; the shift cancels
in normalization. LayerNorm statistics and both residuals stay fp32.

Sharding: core c handles batch c//2, query-token half c%2. Each core gets
its batch's full 2048 tokens (rolled so its own 1024 query tokens come
first) and computes K/V for all of them locally; no collectives.
"""

import numpy as np
import ml_dtypes

import concourse.bacc as bacc
import concourse.tile as tile
from concourse import mybir
from concourse.bass_utils import run_bass_kernel_spmd
from concourse.masks import make_identity

bf16 = mybir.dt.bfloat16
f8 = mybir.dt.float8e4
f32 = mybir.dt.float32
AF = mybir.ActivationFunctionType
ALU = mybir.AluOpType
DR = mybir.MatmulPerfMode.DoubleRow

P = 128
B, T, E, H, D = 4, 2048, 1024, 16, 64
F = 4 * E                    # 4096 MLP hidden
TQ = T // 2                  # 1024 own query tokens per core
NE = E // P                  # 8 e-chunks
NB = NE // 2                 # 4 e-chunk pairs (DoubleRow K blocks)
NPAIR = H // 2               # 8 head pairs
NST = T // P                 # 16 context-token tiles
NSU = NST // 2               # 8 context-token tile pairs
NTS = TQ // P                # 8 own-token tiles
NF = F // P                  # 32 f-chunks
VW = D + 1                   # per-head V width incl. ones column
VP = 80                      # padded V width (16B-aligned fp8 LDW slices)
LN_EPS = 1e-5
SHIFT = 3.0                  # softmax exp shift (cancels in normalization)

_BUILD_CACHE = {}


class _Ctx:
    """Shared build state passed between phase emitters."""
    pass


def _pair3(t, inner):
    """[P, 2*inner_total] tile -> [P, 2, inner_total] AP."""
    return t.rearrange("p (j w) -> p j w", j=2)


def _emit_ln(g, xt, out_bf):
    nc = g.nc
    st = g.stat.tile([P, 2, nc.vector.BN_STATS_DIM], f32, name="bnst")
    xv = xt.rearrange("p (s g) -> p s g", s=2)
    nc.vector.bn_stats(out=st[:, 0, :], in_=xv[:, 0, :])
    nc.vector.bn_stats(out=st[:, 1, :], in_=xv[:, 1, :])
    mv = g.stat.tile([P, nc.vector.BN_AGGR_DIM], f32, name="bnmv")
    nc.vector.bn_aggr(out=mv, in_=st)
    rstd = g.stat.tile([P, 1], f32, name="bnrs")
    nc.scalar.activation(out=rstd, in_=mv[:, 1:2], func=AF.Sqrt, bias=g.eps_t)
    nc.vector.reciprocal(out=rstd, in_=rstd)
    nc.vector.tensor_scalar(
        out=out_bf, in0=xt, scalar1=mv[:, 0:1], scalar2=rstd,
        op0=ALU.subtract, op1=ALU.mult,
    )


def _emit_consts(g):
    nc, consts = g.nc, g.consts
    g.ident = consts.tile([P, P], bf16, name="ident")
    make_identity(nc, g.ident)
    g.eps_t = consts.tile([P, 1], f32, name="eps")
    nc.vector.memset(g.eps_t, LN_EPS)
    g.nshift_t = consts.tile([P, 1], f32, name="nshift")
    nc.vector.memset(g.nshift_t, -SHIFT)
    g.ub_sb = consts.tile([P, NF], f32, name="ubsb")
    nc.sync.dma_start(out=g.ub_sb, in_=g.ub_d[:, :])
    if g.has_qb:
        g.qb_sb = consts.tile([P, NPAIR], f32, name="qbsb")
        nc.sync.dma_start(out=g.qb_sb, in_=g.qb_d[:, :])
        g.kb_sb = consts.tile([P, NPAIR], f32, name="kbsb")
        nc.sync.dma_start(out=g.kb_sb, in_=g.kb_d[:, :])
        g.vb_bc = consts.tile([P, E], bf16, name="vbbc")
        nc.gpsimd.dma_start(
            out=g.vb_bc, in_=g.vbrow_d.ap()[0:1, :].partition_broadcast(P)[:, 0, :]
        )
    if g.has_pb:
        g.pb_bc = consts.tile([P, E], f32, name="pbbc")
        nc.gpsimd.dma_start(
            out=g.pb_bc, in_=g.pbrow_d.ap()[0:1, :].partition_broadcast(P)[:, 0, :]
        )
    if g.has_db:
        g.db_bc = consts.tile([P, E], f32, name="dbbc")
        nc.gpsimd.dma_start(
            out=g.db_bc, in_=g.dbrow_d.ap()[0:1, :].partition_broadcast(P)[:, 0, :]
        )


def _emit_ln1_v(g, xkp, tps, wvp, vps):
    """LN1 + transpose + V projection, software-pipelined: V for token tile
    i-3 is emitted after LN tile i, so the PE's V matmuls never wait on the
    just-issued hT2 copies. hT2 copies alternate Scalar/Vector."""
    nc = g.nc
    LAG = 3
    wv_sb = []
    for b2 in range(NB):
        w = wvp.tile([P, 2 * E], f8, name=f"wv{b2}")
        nc.sync.dma_start(out=w, in_=g.wv_d[b2])
        wv_sb.append(w)
    for u in range(NSU):
        nc.gpsimd.dma_start(
            out=g.va2[u],
            in_=g.vrow_d.ap()[0:1, :].partition_broadcast(P)[:, 0, :],
        )

    def emit_v(s):
        u, sj = s // 2, s % 2
        pv = [vps.tile([P, 512], f32, name="pv") for _ in range(2)]
        scols = slice(s * P, (s + 1) * P)
        for b2 in range(NB):
            lhsT = _pair3(g.hT2[b2], T)[:, :, scols]
            rhs_t = _pair3(wv_sb[b2], E)
            for j in range(2):
                nc.tensor.matmul(
                    pv[j], lhsT, rhs_t[:, :, j * 512:(j + 1) * 512],
                    start=(b2 == 0), stop=(b2 == NB - 1), perf_mode=DR,
                )
        va_v = g.va2[u].rearrange("p (j h c) -> p j h c", j=2, c=VP)
        for j in range(2):
            dst = va_v[:, sj, j * 8:(j + 1) * 8, 0:D]
            src = pv[j].rearrange("p (h d) -> p h d", d=D)
            if g.has_qb:
                vb_view = g.vb_bc.rearrange("p (h d) -> p h d", d=D)[
                    :, j * 8:(j + 1) * 8, :
                ]
                nc.vector.tensor_add(out=dst, in0=src, in1=vb_view)
            else:
                nc.scalar.copy(out=dst, in_=src)

    for i in range(NST):
        xt = xkp.tile([P, E], f32, name="xk")
        nc.sync.dma_start(out=xt, in_=g.xkv_d[i * P:(i + 1) * P, :])
        ht = g.hp.tile([P, E], bf16, name="h")
        _emit_ln(g, xt, ht)
        for c in range(NE):
            tp = tps.tile([P, P], bf16, name="tp")
            nc.tensor.transpose(tp, ht[:, c * P:(c + 1) * P], g.ident)
            dst = g.hT2[c // 2][:, (c % 2) * T + i * P:(c % 2) * T + (i + 1) * P]
            if c % 2 == 0:
                nc.scalar.copy(out=dst, in_=tp)
            else:
                nc.vector.tensor_copy(out=dst, in_=tp)
        if i >= LAG:
            emit_v(i - LAG)
    for s in range(NST - LAG, NST):
        emit_v(s)


def _qk_steps(g, p, qt, kt, wqkp, qkps):
    """Step closures for pair p's Q/K projections (filler for th0 attn)."""
    nc = g.nc
    state = {}

    def s_load():
        state["wq"], state["wk"] = [], []
        for b2 in range(NB):
            wsl = wqkp.tile([P, 2 * P], f8, name="wsl")
            nc.sync.dma_start(out=wsl, in_=g.wq_d[b2, p])
            state["wq"].append(wsl)
        for b2 in range(NB):
            wsl = wqkp.tile([P, 2 * P], f8, name="wsl")
            nc.sync.dma_start(out=wsl, in_=g.wk_d[b2, p])
            state["wk"].append(wsl)

    def s_q(j):
        psq = qkps.tile([P, 512], f32, name="sc")
        for b2 in range(NB):
            nc.tensor.matmul(
                psq, _pair3(state["wq"][b2], P),
                _pair3(g.hT2[b2], T)[:, :, j * 512:(j + 1) * 512],
                start=(b2 == 0), stop=(b2 == NB - 1), perf_mode=DR,
            )
        dst = qt[:, j * 512:(j + 1) * 512]
        if g.has_qb:
            nc.vector.tensor_scalar(
                out=dst, in0=psq, scalar1=g.qb_sb[:, p:p + 1], op0=ALU.add
            )
        else:
            nc.vector.tensor_copy(out=dst, in_=psq)

    def s_k(blk):
        s0 = blk * 512
        psk = qkps.tile([P, 512], f32, name="sc")
        for b2 in range(NB):
            nc.tensor.matmul(
                psk, _pair3(state["wk"][b2], P),
                _pair3(g.hT2[b2], T)[:, :, s0:s0 + 512],
                start=(b2 == 0), stop=(b2 == NB - 1), perf_mode=DR,
            )
        dst = kt[:, s0:s0 + 512]
        if g.has_qb:
            nc.vector.tensor_scalar(
                out=dst, in0=psk, scalar1=g.kb_sb[:, p:p + 1], op0=ALU.add
            )
        else:
            nc.vector.tensor_copy(out=dst, in_=psk)

    return ([s_load] + [(lambda j=j: s_q(j)) for j in range(2)]
            + [(lambda b=b: s_k(b)) for b in range(4)])


def _emit_qkt_pair(g, p, qt, kt, wqkp, qkps):
    """Q^T and K^T for head pair p: [128 (2 heads x 64d), tokens], bf16.
    fp8 DoubleRow over e-chunk pairs."""
    nc = g.nc
    wq_sb = []
    for b2 in range(NB):
        wsl = wqkp.tile([P, 2 * P], f8, name="wsl")
        nc.sync.dma_start(out=wsl, in_=g.wq_d[b2, p])
        wq_sb.append(wsl)
    for j in range(2):
        psq = qkps.tile([P, 512], f32, name="sc")
        for b2 in range(NB):
            nc.tensor.matmul(
                psq, _pair3(wq_sb[b2], P),
                _pair3(g.hT2[b2], T)[:, :, j * 512:(j + 1) * 512],
                start=(b2 == 0), stop=(b2 == NB - 1), perf_mode=DR,
            )
        dst = qt[:, j * 512:(j + 1) * 512]
        if g.has_qb:
            nc.vector.tensor_scalar(
                out=dst, in0=psq, scalar1=g.qb_sb[:, p:p + 1], op0=ALU.add
            )
        else:
            nc.vector.tensor_copy(out=dst, in_=psq)
    wk_sb = []
    for b2 in range(NB):
        wsl = wqkp.tile([P, 2 * P], f8, name="wsl")
        nc.sync.dma_start(out=wsl, in_=g.wk_d[b2, p])
        wk_sb.append(wsl)
    for blk in range(4):
        s0 = blk * 512
        psk = qkps.tile([P, 512], f32, name="sc")
        for b2 in range(NB):
            nc.tensor.matmul(
                psk, _pair3(wk_sb[b2], P),
                _pair3(g.hT2[b2], T)[:, :, s0:s0 + 512],
                start=(b2 == 0), stop=(b2 == NB - 1), perf_mode=DR,
            )
        dst = kt[:, s0:s0 + 512]
        if g.has_qb:
            nc.vector.tensor_scalar(
                out=dst, in0=psk, scalar1=g.kb_sb[:, p:p + 1], op0=ALU.add
            )
        else:
            nc.vector.tensor_copy(out=dst, in_=psk)


def _emit_attn_pair(g, p, th, qt, kt, ptp, smp, scps, atps, filler=None):
    """Scores (fp8 operands, transposed), shifted exp to fp8, attn^T via
    DoubleRow + softmax denom (ones column), normalize -> catT2 (fp8).
    `filler()` (if given) is called once per context-tile pair to emit
    PE work that runs while the Act engine chews through the exps."""
    nc = g.nc
    u2, j2 = p // 2, p % 2
    tcols = slice(th * 512, (th + 1) * 512)
    at0 = atps.tile([D + 1, 512], f32, name="ps0")
    at1 = atps.tile([D + 1, 512], f32, name="ps1")
    for u in range(NSU):
        pta = ptp.tile([P, 1024], f8, name="pta")
        ptb = ptp.tile([P, 1024], f8, name="ptb")
        for sj in range(2):
            s = 2 * u + sj
            scols = slice(s * P, (s + 1) * P)
            sc0 = scps.tile([P, 512], f32, name="sc")
            sc1 = scps.tile([P, 512], f32, name="sc")
            # S^T[s,t] = (K^T slice).T @ Q^T slice; the two heads live
            # on row-groups 0-63 / 64-127 so the matmuls pack.
            nc.tensor.matmul(sc0, kt[0:D, scols], qt[0:D, tcols],
                             start=True, stop=True)
            nc.tensor.matmul(sc1, kt[D:2 * D, scols], qt[D:2 * D, tcols],
                             start=True, stop=True)
            nc.scalar.activation(out=pta[:, sj * 512:(sj + 1) * 512],
                                 in_=sc0, func=AF.Exp, bias=g.nshift_t)
            nc.scalar.activation(out=ptb[:, sj * 512:(sj + 1) * 512],
                                 in_=sc1, func=AF.Exp, bias=g.nshift_t)
        if filler is not None:
            filler()
        va_v = _pair3(g.va2[u], H * VP)
        nc.tensor.matmul(
            at0, va_v[:, :, (2 * p) * VP:(2 * p) * VP + VW],
            _pair3(pta, 512),
            start=(u == 0), stop=(u == NSU - 1), perf_mode=DR,
        )
        nc.tensor.matmul(
            at1, va_v[:, :, (2 * p + 1) * VP:(2 * p + 1) * VP + VW],
            _pair3(ptb, 512),
            start=(u == 0), stop=(u == NSU - 1), perf_mode=DR,
        )
    se0 = smp.tile([1, 512], f32, name="se0")
    se1 = smp.tile([1, 512], f32, name="se1")
    dn0 = smp.tile([1, 512], f32, name="dn0")
    dn1 = smp.tile([1, 512], f32, name="dn1")
    # recip_approx_fast misreads PSUM on HW -- bounce rows to SBUF first
    nc.vector.tensor_copy(out=dn0, in_=at0[D:D + 1, :])
    nc.vector.tensor_copy(out=dn1, in_=at1[D:D + 1, :])
    nc.vector.reciprocal_approx_fast(out=se0, in_=dn0)
    nc.vector.reciprocal_approx_fast(out=se1, in_=dn1)
    rb0 = smp.tile([D, 512], f32, name="rb0")
    rb1 = smp.tile([D, 512], f32, name="rb1")
    nc.gpsimd.partition_broadcast(rb0, se0)
    nc.gpsimd.partition_broadcast(rb1, se1)
    c0 = j2 * TQ + th * 512
    nc.vector.tensor_mul(out=g.catT2[u2][0:D, c0:c0 + 512],
                         in0=at0[0:D, :], in1=rb0)
    nc.vector.tensor_mul(out=g.catT2[u2][D:2 * D, c0:c0 + 512],
                         in0=at1[0:D, :], in1=rb1)


def _emit_proj_ln2(g, th, xq2p, h2p, pps, t2ps):
    nc = g.nc
    pw_sb = g.pw_sb
    for ts in range(th * 4, th * 4 + 4):
        trows = slice(ts * P, (ts + 1) * P)
        xres = xq2p.tile([P, E], bf16, name="xres")
        nc.gpsimd.dma_start(out=xres, in_=g.xkv_d[ts * P:(ts + 1) * P, :])
        psy = [pps.tile([P, 512], f32, name=f"py{j}") for j in range(2)]
        for u in range(NB):
            lhsT = _pair3(g.catT2[u], TQ)[:, :, trows]
            rhs_t = _pair3(pw_sb[u], E)
            for j in range(2):
                nc.tensor.matmul(
                    psy[j], lhsT, rhs_t[:, :, j * 512:(j + 1) * 512],
                    start=(u == 0), stop=(u == NB - 1), perf_mode=DR,
                )
        x2 = xq2p.tile([P, E], bf16, name="x2t")
        for j in range(2):
            jc = slice(j * 512, (j + 1) * 512)
            if g.has_pb:
                nc.vector.tensor_add(out=x2[:, jc], in0=psy[j], in1=g.pb_bc[:, jc])
                nc.vector.tensor_add(out=x2[:, jc], in0=x2[:, jc],
                                     in1=xres[:, jc])
            else:
                nc.vector.tensor_add(out=x2[:, jc], in0=psy[j],
                                     in1=xres[:, jc])
        nc.sync.dma_start(out=g.x2_d[ts * P:(ts + 1) * P, :], in_=x2)
        h2 = h2p.tile([P, E], bf16, name="h2")
        _emit_ln(g, x2, h2)
        for c in range(NE):
            tp = t2ps.tile([P, P], bf16, name="t2")
            nc.tensor.transpose(tp, h2[:, c * P:(c + 1) * P], g.ident)
            nc.scalar.copy(out=g.h2T[c][:, trows], in_=tp)


TQQ = 256  # token quarter


def _mlp_steps(g, q, pup, dnp, dwps, outp):
    """Closure list for quarter q: 32 up steps, then per e-half 32 down
    steps + a finish. Each step is ~0.5-1.7us of PE work, emitted as
    attention filler so it runs under the Act engine's exp stream."""
    nc = g.nc
    qcols = slice(q * TQQ, (q + 1) * TQQ)
    state = {}

    def up_step(f):
        pu = pup.tile([P, TQQ], f32, name="pu")
        for c in range(NE):
            nc.tensor.matmul(
                pu, g.uw_sb[c][:, f * P:(f + 1) * P], g.h2T[c][:, qcols],
                start=(c == 0), stop=(c == NE - 1),
            )
        nc.scalar.activation(out=g.hid_tiles[f], in_=pu, func=AF.Relu,
                             bias=g.ub_sb[:, f:f + 1])

    def down_step(j, f):
        jc = slice(j * 512, (j + 1) * 512)
        if f == 0:
            state["dn"] = [dnp.tile([P, 512], f32, name=f"dnq{t2}")
                           for t2 in range(2)]
        dn = state["dn"]
        dwt = dwps.tile([P, 512], bf16, name="dwt")
        nc.sync.dma_start(out=dwt, in_=g.dw_d[f][:, jc])
        for t2 in range(2):
            nc.tensor.matmul(
                dn[t2], g.hid_tiles[f][:, t2 * P:(t2 + 1) * P], dwt,
                start=(f == 0), stop=(f == NF - 1),
            )

    def finish(j):
        jc = slice(j * 512, (j + 1) * 512)
        dn = state["dn"]
        for t2 in range(2):
            ti = q * 2 + t2
            x2s = outp.tile([P, 512], bf16, name="x2s")
            nc.gpsimd.dma_start(out=x2s, in_=g.x2_d[ti * P:(ti + 1) * P, jc])
            ot = outp.tile([P, 512], f32, name="ot")
            if g.has_db:
                nc.vector.tensor_add(out=ot, in0=dn[t2], in1=g.db_bc[:, jc])
                nc.vector.tensor_add(out=ot, in0=ot, in1=x2s)
            else:
                nc.vector.tensor_add(out=ot, in0=dn[t2], in1=x2s)
            nc.sync.dma_start(out=g.out_d[ti * P:(ti + 1) * P, jc], in_=ot)

    steps = [(lambda f=f: up_step(f)) for f in range(NF)]
    for j in range(2):
        steps += [(lambda j=j, f=f: down_step(j, f)) for f in range(NF)]
        steps.append(lambda j=j: finish(j))
    return steps


def _emit_mlp_quarter_wide(g, q, hidp, dwps, outp, upps, dnps):
    """Tail variant: f-outer down with full-width dw loads and 2x[P,E] dn."""
    nc = g.nc
    qcols = slice(q * TQQ, (q + 1) * TQQ)
    dn = [dnps.tile([P, E], f32, name=f"dnw{j}") for j in range(2)]
    for f in range(NF):
        pu = upps.tile([P, TQQ], f32, name="puw")
        for c in range(NE):
            nc.tensor.matmul(
                pu, g.uw_sb[c][:, f * P:(f + 1) * P], g.h2T[c][:, qcols],
                start=(c == 0), stop=(c == NE - 1),
            )
        hid = g.hid_tiles[f]
        nc.scalar.activation(out=hid, in_=pu, func=AF.Relu,
                             bias=g.ub_sb[:, f:f + 1])
        dwt = dwps.tile([P, E], bf16, name="dwf")
        nc.sync.dma_start(out=dwt, in_=g.dw_d[f])
        for t2 in range(2):
            for j in range(2):
                nc.tensor.matmul(
                    dn[t2][:, j * 512:(j + 1) * 512],
                    hid[:, t2 * P:(t2 + 1) * P],
                    dwt[:, j * 512:(j + 1) * 512],
                    start=(f == 0), stop=(f == NF - 1),
                )
    for t2 in range(2):
        ti = q * 2 + t2
        x2s = outp.tile([P, E], bf16, name="x2w")
        nc.gpsimd.dma_start(out=x2s, in_=g.x2_d[ti * P:(ti + 1) * P, :])
        ot = outp.tile([P, E], f32, name="otw")
        if g.has_db:
            nc.vector.tensor_add(out=ot, in0=dn[t2], in1=g.db_bc)
            nc.vector.tensor_add(out=ot, in0=ot, in1=x2s)
        else:
            nc.vector.tensor_add(out=ot, in0=dn[t2], in1=x2s)
        nc.sync.dma_start(out=g.out_d[ti * P:(ti + 1) * P, :], in_=ot)





def _build(flags, reps=1):
    has_qb, has_pb, has_db = flags
    nc = bacc.Bacc("TRN2", target_bir_lowering=False, debug=False, num_devices=8)

    g = _Ctx()
    g.nc = nc
    g.has_qb, g.has_pb, g.has_db = flags
    g.xkv_d = nc.dram_tensor("xkv", [T, E], f32, kind="ExternalInput")
    g.wq_d = nc.dram_tensor("wq", [NB, NPAIR, P, 2 * P], f8, kind="ExternalInput")
    g.wk_d = nc.dram_tensor("wk", [NB, NPAIR, P, 2 * P], f8, kind="ExternalInput")
    g.wv_d = nc.dram_tensor("wv", [NB, P, 2 * E], f8, kind="ExternalInput")
    g.vrow_d = nc.dram_tensor("vrow", [1, 2 * H * VP], f8, kind="ExternalInput")
    g.pw_d = nc.dram_tensor("pw", [NB, P, 2 * E], f8, kind="ExternalInput")
    g.uw_d = nc.dram_tensor("uw", [NE, P, F], bf16, kind="ExternalInput")
    g.ub_d = nc.dram_tensor("ub", [P, NF], f32, kind="ExternalInput")
    g.dw_d = nc.dram_tensor("dw", [NF, P, E], bf16, kind="ExternalInput")
    if has_qb:
        g.qb_d = nc.dram_tensor("qb", [P, NPAIR], f32, kind="ExternalInput")
        g.kb_d = nc.dram_tensor("kb", [P, NPAIR], f32, kind="ExternalInput")
        g.vbrow_d = nc.dram_tensor("vbrow", [1, E], bf16, kind="ExternalInput")
    if has_pb:
        g.pbrow_d = nc.dram_tensor("pbrow", [1, E], f32, kind="ExternalInput")
    if has_db:
        g.dbrow_d = nc.dram_tensor("dbrow", [1, E], f32, kind="ExternalInput")
    g.x2_d = nc.dram_tensor("x2s", [TQ, E], bf16, kind="Internal")
    g.out_d = nc.dram_tensor("out", [TQ, E], f32, kind="ExternalOutput")

    with tile.TileContext(nc) as tc:
        with (
            tc.tile_pool(name="consts", bufs=1) as consts,
            tc.tile_pool(name="stat", bufs=4) as stat,
            tc.tile_pool(name="catp", bufs=1) as catp,
            tc.tile_pool(name="x2p", bufs=1) as x2p,
            tc.tile_pool(name="h2Tp", bufs=1) as h2Tp,
        ):
            g.consts, g.stat = consts, stat
            _emit_consts(g)
            for _rep in range(reps):
                _emit_all(g, tc, catp, x2p, h2Tp)

    nc.finalize()
    return nc


def _emit_all(g, tc, catp, x2p, h2Tp):
    g.catT2 = [catp.tile([P, 2 * TQ], f8, name=f"catT{u}") for u in range(NB)]
    g.h2T = [h2Tp.tile([P, TQ], bf16, name=f"h2T{c}") for c in range(NE)]

    g.pwp = tc.alloc_tile_pool(name="pwp", bufs=1)
    g.uwp = tc.alloc_tile_pool(name="uwp", bufs=1)
    hidp = tc.alloc_tile_pool(name="hidp", bufs=1)
    g.hid_tiles = [hidp.tile([P, TQQ], bf16, name=f"hid{f}")
                   for f in range(NF)]

    with tc.tile_pool(name="vaug", bufs=1) as vap:
        g.va2 = [vap.tile([P, 2 * H * VP], f8, name=f"va{u}")
                 for u in range(NSU)]
        qktp = tc.alloc_tile_pool(name="qktp", bufs=1)
        qts = [qktp.tile([P, TQ], f8, name=f"qt{p}") for p in range(NPAIR)]
        kts = [qktp.tile([P, T], f8, name=f"kt{p}") for p in range(NPAIR)]

        with tc.tile_pool(name="hTp", bufs=1) as hTp:
            g.hT2 = [hTp.tile([P, 2 * T], f8, name=f"hT{b}") for b in range(NB)]
            with (
                tc.tile_pool(name="hp", bufs=4) as hp,
                tc.tile_pool(name="xk", bufs=3) as xkp,
                tc.tile_pool(name="tps", bufs=4, space="PSUM") as tps,
                tc.tile_pool(name="wvp", bufs=1) as wvp,
                tc.tile_pool(name="vps", bufs=2, space="PSUM") as vps,
            ):
                g.hp = hp
                _emit_ln1_v(g, xkp, tps, wvp, vps)

            g.pw_sb = []
            for u in range(NB):
                w = g.pwp.tile([P, 2 * E], f8, name=f"pw{u}")
                g.nc.gpsimd.dma_start(out=w, in_=g.pw_d[u])
                g.pw_sb.append(w)

            # th=0 attention; Q/K for pair p+1 emitted as PE filler under
            # pair p's exp stream (pair 0's Q/K runs upfront)
            with (
                tc.tile_pool(name="ptp0", bufs=3) as ptp0,
                tc.tile_pool(name="smp0", bufs=1) as smp0,
                tc.tile_pool(name="wqk", bufs=6) as wqkp,
                tc.tile_pool(name="atps", bufs=1, space="PSUM") as atps,
                tc.tile_pool(name="scps", bufs=6, space="PSUM") as scps,
            ):
                for st in _qk_steps(g, 0, qts[0], kts[0], wqkp, scps):
                    st()
                qk_queue = []
                qi = {"i": 0}

                def qk_filler(n=1):
                    for _ in range(n):
                        if qi["i"] < len(qk_queue):
                            qk_queue[qi["i"]]()
                            qi["i"] += 1

                for p in range(NPAIR):
                    if p + 1 < NPAIR:
                        qk_queue.extend(
                            _qk_steps(g, p + 1, qts[p + 1], kts[p + 1],
                                      wqkp, scps))
                    _emit_attn_pair(g, p, 0, qts[p], kts[p], ptp0, smp0,
                                    scps, atps, filler=qk_filler)
                qk_filler(len(qk_queue))

        # hT2 freed; proj th0, then th1 attention with MLP q0/q1 as filler
        with (
            tc.tile_pool(name="ptp", bufs=3) as ptp,
            tc.tile_pool(name="smp", bufs=1) as smp,
            tc.tile_pool(name="xq2", bufs=2) as xq2p,
            tc.tile_pool(name="h2p", bufs=2) as h2p,
            tc.tile_pool(name="dwps", bufs=4) as dwps,
            tc.tile_pool(name="outp", bufs=2) as outp,
        ):
            g.uw_sb = []
            for c in range(NE):
                w = g.uwp.tile([P, F], bf16, name=f"uw{c}")
                g.nc.gpsimd.dma_start(out=w, in_=g.uw_d[c])
                g.uw_sb.append(w)
            with (
                tc.tile_pool(name="pps", bufs=2, space="PSUM") as pps,
                tc.tile_pool(name="t2ps", bufs=2, space="PSUM") as t2ps,
            ):
                _emit_proj_ln2(g, 0, xq2p, h2p, pps, t2ps)
            with (
                tc.tile_pool(name="atps2", bufs=1, space="PSUM") as atps2,
                tc.tile_pool(name="scps2", bufs=2, space="PSUM") as scps2,
                tc.tile_pool(name="pup", bufs=2, space="PSUM") as pup,
                tc.tile_pool(name="dnp", bufs=1, space="PSUM") as dnp,
            ):
                steps = (_mlp_steps(g, 0, pup, dnp, dwps, outp)
                         + _mlp_steps(g, 1, pup, dnp, dwps, outp))
                si = {"i": 0}

                def filler(n=3):
                    for _ in range(n):
                        if si["i"] < len(steps):
                            steps[si["i"]]()
                            si["i"] += 1

                for p in range(NPAIR):
                    _emit_attn_pair(g, p, 1, qts[p], kts[p], ptp, smp,
                                    scps2, atps2, filler=filler)
                filler(len(steps))  # flush leftovers
            with (
                tc.tile_pool(name="pps2", bufs=2, space="PSUM") as pps2,
                tc.tile_pool(name="t2ps2", bufs=2, space="PSUM") as t2ps2,
            ):
                _emit_proj_ln2(g, 1, xq2p, h2p, pps2, t2ps2)
        qktp.release()

    # MLP tail: th1's quarters, wide down (attention PSUM/SBUF freed)
    with (
        tc.tile_pool(name="uppsT", bufs=2, space="PSUM") as uppsT,
        tc.tile_pool(name="dnpsT", bufs=1, space="PSUM") as dnpsT,
        tc.tile_pool(name="dwpsT", bufs=4) as dwpsT,
        tc.tile_pool(name="outpT", bufs=2) as outpT,
    ):
        for q in (2, 3):
            _emit_mlp_quarter_wide(g, q, hidp, dwpsT, outpT, uppsT, dnpsT)
    hidp.release()
    g.uwp.release()
    g.pwp.release()


def _get_nc(flags, reps=1):
    key = (flags, reps)
    if key not in _BUILD_CACHE:
        _BUILD_CACHE[key] = _build(flags, reps)
    return _BUILD_CACHE[key]


def _q8(x):
    return np.clip(np.asarray(x, np.float32), -240.0, 240.0).astype(
        ml_dtypes.float8_e4m3)


def _prep(x, Wq, Wk, Wv, proj_w, proj_b, ln1_g, ln1_b, ln2_g, ln2_b,
          up_w, up_b, down_w, down_b):
    """Host-side shard + weight fold/cast/layout. Returns (flags, in_maps)."""
    bfl = ml_dtypes.bfloat16
    x = np.ascontiguousarray(np.asarray(x, dtype=np.float32))
    Wq = np.asarray(Wq, np.float32)
    Wk = np.asarray(Wk, np.float32)
    Wv = np.asarray(Wv, np.float32)
    g1 = np.asarray(ln1_g, np.float32)
    b1 = np.asarray(ln1_b, np.float32)
    g2 = np.asarray(ln2_g, np.float32)
    b2 = np.asarray(ln2_b, np.float32)
    proj_w = np.asarray(proj_w, np.float32)
    up_w = np.asarray(up_w, np.float32)
    down_w = np.asarray(down_w, np.float32)

    # [H, E, D] -> [E, H*D]; fold attention scale into Q, LN1 gain into all
    wq_all = (Wq * (D ** -0.5)).transpose(1, 0, 2).reshape(E, E)
    wk_all = Wk.transpose(1, 0, 2).reshape(E, E)
    wv_all = Wv.transpose(1, 0, 2).reshape(E, E)
    qb_vec = b1 @ wq_all
    kb_vec = b1 @ wk_all
    vb_vec = b1 @ wv_all
    wq_f = g1[:, None] * wq_all
    wk_f = g1[:, None] * wk_all
    wv_f = g1[:, None] * wv_all

    def _pair_chunks_qk(w):  # [E, E] -> [NB, NPAIR, P, 2P], DR pair layout
        return np.ascontiguousarray(
            _q8(w.reshape(NB, 2, P, NPAIR, P).transpose(0, 3, 2, 1, 4)
                .reshape(NB, NPAIR, P, 2 * P)))

    def _pair_rows(w, ncols):  # [E_in, ncols] -> [E_in/256, P, 2*ncols]
        nb = w.shape[0] // 256
        return np.ascontiguousarray(
            _q8(w.reshape(nb, 2, P, ncols).transpose(0, 2, 1, 3)
                .reshape(nb, P, 2 * ncols)))

    vrow = np.zeros((1, 2 * H * VP), np.float32)
    vrow.reshape(2, H, VP)[:, :, D] = 1.0

    uw_f = g2[:, None] * up_w
    ub_f = np.asarray(up_b, np.float32) + b2 @ up_w

    has_qb = bool(np.any(b1 != 0))
    has_pb = bool(np.any(np.asarray(proj_b) != 0))
    has_db = bool(np.any(np.asarray(down_b) != 0))
    flags = (has_qb, has_pb, has_db)

    shared = {
        "wq": _pair_chunks_qk(wq_f),
        "wk": _pair_chunks_qk(wk_f),
        "wv": _pair_rows(wv_f, E),
        "vrow": _q8(vrow),
        "pw": _pair_rows(proj_w, E),
        "uw": np.ascontiguousarray(uw_f.reshape(NE, P, F).astype(bfl)),
        "ub": np.ascontiguousarray(ub_f.reshape(NF, P).T.astype(np.float32)),
        "dw": np.ascontiguousarray(down_w.reshape(NF, P, E).astype(bfl)),
    }
    if has_qb:
        shared["qb"] = np.ascontiguousarray(
            qb_vec.reshape(NPAIR, P).T.astype(np.float32))
        shared["kb"] = np.ascontiguousarray(
            kb_vec.reshape(NPAIR, P).T.astype(np.float32))
        shared["vbrow"] = vb_vec.reshape(1, E).astype(bfl)
    if has_pb:
        shared["pbrow"] = np.asarray(proj_b, np.float32).reshape(1, E)
    if has_db:
        shared["dbrow"] = np.asarray(down_b, np.float32).reshape(1, E)

    in_maps = []
    for c in range(8):
        b, half = c // 2, c % 2
        xb = x[b]
        if half == 1:
            xb = np.concatenate([xb[TQ:], xb[:TQ]], axis=0)
        in_maps.append({"xkv": np.ascontiguousarray(xb), **shared})
    return flags, in_maps


def kernel(**inputs) -> np.ndarray:
    flags, in_maps = _prep(**inputs)
    nc = _get_nc(flags)
    res = run_bass_kernel_spmd(nc, in_maps, core_ids=list(range(8)))
    out = np.empty((B, T, E), np.float32)
    for c in range(8):
        b, half = c // 2, c % 2
        out[b, half * TQ:(half + 1) * TQ, :] = res.results[c]["out"]
    return out


# revision 7
# speedup vs baseline: 1.0144x; 1.0144x over previous
"""Trainium2 Bass kernel for a dense transformer block (nn_Block_120259084502).

fp8 DoubleRow variant: attention matmuls (Q/K/V projections, attn@V, output
proj) run in fp8-e4m3 with DoubleRow perf mode (2 contraction elements per
cycle -> half the matmul streams). Scores (K=64/head, no DR gain) and the
MLP (fp8 fails the error budget there) stay bf16. Softmax runs shifted
(exp(S-2)) so probabilities fit fp8-e4m3's +/-240 range; the shift cancels
in normalization. LayerNorm statistics and both residuals stay fp32.

Sharding: core c handles batch c//2, query-token half c%2. Each core gets
its batch's full 2048 tokens (rolled so its own 1024 query tokens come
first) and computes K/V for all of them locally; no collectives.
"""

import numpy as np
import ml_dtypes

import concourse.bacc as bacc
import concourse.tile as tile
from concourse import mybir
from concourse.bass_utils import run_bass_kernel_spmd
from concourse.masks import make_identity

bf16 = mybir.dt.bfloat16
f8 = mybir.dt.float8e4
f32 = mybir.dt.float32
AF = mybir.ActivationFunctionType
ALU = mybir.AluOpType
DR = mybir.MatmulPerfMode.DoubleRow

P = 128
B, T, E, H, D = 4, 2048, 1024, 16, 64
F = 4 * E                    # 4096 MLP hidden
TQ = T // 2                  # 1024 own query tokens per core
NE = E // P                  # 8 e-chunks
NB = NE // 2                 # 4 e-chunk pairs (DoubleRow K blocks)
NPAIR = H // 2               # 8 head pairs
NST = T // P                 # 16 context-token tiles
NSU = NST // 2               # 8 context-token tile pairs
NTS = TQ // P                # 8 own-token tiles
NF = F // P                  # 32 f-chunks
VW = D + 1                   # per-head V width incl. ones column
VP = 80                      # padded V width (16B-aligned fp8 LDW slices)
LN_EPS = 1e-5
SHIFT = 3.0                  # softmax exp shift (cancels in normalization)

_BUILD_CACHE = {}


class _Ctx:
    """Shared build state passed between phase emitters."""
    pass


def _pair3(t, inner):
    """[P, 2*inner_total] tile -> [P, 2, inner_total] AP."""
    return t.rearrange("p (j w) -> p j w", j=2)


def _emit_ln(g, xt, out_bf):
    nc = g.nc
    st = g.stat.tile([P, 2, nc.vector.BN_STATS_DIM], f32, name="bnst")
    xv = xt.rearrange("p (s g) -> p s g", s=2)
    nc.vector.bn_stats(out=st[:, 0, :], in_=xv[:, 0, :])
    nc.vector.bn_stats(out=st[:, 1, :], in_=xv[:, 1, :])
    mv = g.stat.tile([P, nc.vector.BN_AGGR_DIM], f32, name="bnmv")
    nc.vector.bn_aggr(out=mv, in_=st)
    rstd = g.stat.tile([P, 1], f32, name="bnrs")
    nc.scalar.activation(out=rstd, in_=mv[:, 1:2], func=AF.Sqrt, bias=g.eps_t)
    nc.vector.reciprocal(out=rstd, in_=rstd)
    nc.vector.tensor_scalar(
        out=out_bf, in0=xt, scalar1=mv[:, 0:1], scalar2=rstd,
        op0=ALU.subtract, op1=ALU.mult,
    )


def _emit_consts(g):
    nc, consts = g.nc, g.consts
    g.ident = consts.tile([P, P], bf16, name="ident")
    make_identity(nc, g.ident)
    g.eps_t = consts.tile([P, 1], f32, name="eps")
    nc.vector.memset(g.eps_t, LN_EPS)
    g.nshift_t = consts.tile([P, 1], f32, name="nshift")
    nc.vector.memset(g.nshift_t, -SHIFT)
    g.ub_sb = consts.tile([P, NF], f32, name="ubsb")
    nc.sync.dma_start(out=g.ub_sb, in_=g.ub_d[:, :])
    if g.has_qb:
        g.qb_sb = consts.tile([P, NPAIR], f32, name="qbsb")
        nc.sync.dma_start(out=g.qb_sb, in_=g.qb_d[:, :])
        g.kb_sb = consts.tile([P, NPAIR], f32, name="kbsb")
        nc.sync.dma_start(out=g.kb_sb, in_=g.kb_d[:, :])
        g.vb_bc = consts.tile([P, E], bf16, name="vbbc")
        nc.gpsimd.dma_start(
            out=g.vb_bc, in_=g.vbrow_d.ap()[0:1, :].partition_broadcast(P)[:, 0, :]
        )
    if g.has_pb:
        g.pb_bc = consts.tile([P, E], f32, name="pbbc")
        nc.gpsimd.dma_start(
            out=g.pb_bc, in_=g.pbrow_d.ap()[0:1, :].partition_broadcast(P)[:, 0, :]
        )
    if g.has_db:
        g.db_bc = consts.tile([P, E], f32, name="dbbc")
        nc.gpsimd.dma_start(
            out=g.db_bc, in_=g.dbrow_d.ap()[0:1, :].partition_broadcast(P)[:, 0, :]
        )


def _emit_ln1_v(g, xkp, tps, wvp, vps):
    """LN1 + transpose + V projection, software-pipelined: V for token tile
    i-3 is emitted after LN tile i, so the PE's V matmuls never wait on the
    just-issued hT2 copies. hT2 copies alternate Scalar/Vector."""
    nc = g.nc
    LAG = 3
    wv_sb = []
    for b2 in range(NB):
        w = wvp.tile([P, 2 * E], f8, name=f"wv{b2}")
        nc.sync.dma_start(out=w, in_=g.wv_d[b2])
        wv_sb.append(w)
    for u in range(NSU):
        nc.gpsimd.dma_start(
            out=g.va2[u],
            in_=g.vrow_d.ap()[0:1, :].partition_broadcast(P)[:, 0, :],
        )

    def emit_v(s):
        u, sj = s // 2, s % 2
        pv = [vps.tile([P, 512], f32, name="pv") for _ in range(2)]
        scols = slice(s * P, (s + 1) * P)
        for b2 in range(NB):
            lhsT = _pair3(g.hT2[b2], T)[:, :, scols]
            rhs_t = _pair3(wv_sb[b2], E)
            for j in range(2):
                nc.tensor.matmul(
                    pv[j], lhsT, rhs_t[:, :, j * 512:(j + 1) * 512],
                    start=(b2 == 0), stop=(b2 == NB - 1), perf_mode=DR,
                )
        va_v = g.va2[u].rearrange("p (j h c) -> p j h c", j=2, c=VP)
        for j in range(2):
            dst = va_v[:, sj, j * 8:(j + 1) * 8, 0:D]
            src = pv[j].rearrange("p (h d) -> p h d", d=D)
            if g.has_qb:
                vb_view = g.vb_bc.rearrange("p (h d) -> p h d", d=D)[
                    :, j * 8:(j + 1) * 8, :
                ]
                nc.vector.tensor_add(out=dst, in0=src, in1=vb_view)
            else:
                nc.scalar.copy(out=dst, in_=src)

    for i in range(NST):
        xt = xkp.tile([P, E], f32, name="xk")
        nc.sync.dma_start(out=xt, in_=g.xkv_d[i * P:(i + 1) * P, :])
        ht = g.hp.tile([P, E], bf16, name="h")
        _emit_ln(g, xt, ht)
        for c in range(NE):
            tp = tps.tile([P, P], bf16, name="tp")
            nc.tensor.transpose(tp, ht[:, c * P:(c + 1) * P], g.ident)
            dst = g.hT2[c // 2][:, (c % 2) * T + i * P:(c % 2) * T + (i + 1) * P]
            if c % 2 == 0:
                nc.scalar.copy(out=dst, in_=tp)
            else:
                nc.vector.tensor_copy(out=dst, in_=tp)
        if i >= LAG:
            emit_v(i - LAG)
    for s in range(NST - LAG, NST):
        emit_v(s)


def _qk_steps(g, p, qt, kt, wqkp, qkps):
    """Step closures for pair p's Q/K projections (filler for th0 attn)."""
    nc = g.nc
    state = {}

    def s_load():
        state["wq"], state["wk"] = [], []
        for b2 in range(NB):
            wsl = wqkp.tile([P, 2 * P], f8, name="wsl")
            nc.sync.dma_start(out=wsl, in_=g.wq_d[b2, p])
            state["wq"].append(wsl)
        for b2 in range(NB):
            wsl = wqkp.tile([P, 2 * P], f8, name="wsl")
            nc.sync.dma_start(out=wsl, in_=g.wk_d[b2, p])
            state["wk"].append(wsl)

    def s_q(j):
        psq = qkps.tile([P, 512], f32, name="sc")
        for b2 in range(NB):
            nc.tensor.matmul(
                psq, _pair3(state["wq"][b2], P),
                _pair3(g.hT2[b2], T)[:, :, j * 512:(j + 1) * 512],
                start=(b2 == 0), stop=(b2 == NB - 1), perf_mode=DR,
            )
        dst = qt[:, j * 512:(j + 1) * 512]
        if g.has_qb:
            nc.vector.tensor_scalar(
                out=dst, in0=psq, scalar1=g.qb_sb[:, p:p + 1], op0=ALU.add
            )
        else:
            nc.vector.tensor_copy(out=dst, in_=psq)

    def s_k(blk):
        s0 = blk * 512
        psk = qkps.tile([P, 512], f32, name="sc")
        for b2 in range(NB):
            nc.tensor.matmul(
                psk, _pair3(state["wk"][b2], P),
                _pair3(g.hT2[b2], T)[:, :, s0:s0 + 512],
                start=(b2 == 0), stop=(b2 == NB - 1), perf_mode=DR,
            )
        dst = kt[:, s0:s0 + 512]
        if g.has_qb:
            nc.vector.tensor_scalar(
                out=dst, in0=psk, scalar1=g.kb_sb[:, p:p + 1], op0=ALU.add
            )
        else:
            nc.vector.tensor_copy(out=dst, in_=psk)

    return ([s_load] + [(lambda j=j: s_q(j)) for j in range(2)]
            + [(lambda b=b: s_k(b)) for b in range(4)])


def _emit_qkt_pair(g, p, qt, kt, wqkp, qkps):
    """Q^T and K^T for head pair p: [128 (2 heads x 64d), tokens], bf16.
    fp8 DoubleRow over e-chunk pairs."""
    nc = g.nc
    wq_sb = []
    for b2 in range(NB):
        wsl = wqkp.tile([P, 2 * P], f8, name="wsl")
        nc.sync.dma_start(out=wsl, in_=g.wq_d[b2, p])
        wq_sb.append(wsl)
    for j in range(2):
        psq = qkps.tile([P, 512], f32, name="sc")
        for b2 in range(NB):
            nc.tensor.matmul(
                psq, _pair3(wq_sb[b2], P),
                _pair3(g.hT2[b2], T)[:, :, j * 512:(j + 1) * 512],
                start=(b2 == 0), stop=(b2 == NB - 1), perf_mode=DR,
            )
        dst = qt[:, j * 512:(j + 1) * 512]
        if g.has_qb:
            nc.vector.tensor_scalar(
                out=dst, in0=psq, scalar1=g.qb_sb[:, p:p + 1], op0=ALU.add
            )
        else:
            nc.vector.tensor_copy(out=dst, in_=psq)
    wk_sb = []
    for b2 in range(NB):
        wsl = wqkp.tile([P, 2 * P], f8, name="wsl")
        nc.sync.dma_start(out=wsl, in_=g.wk_d[b2, p])
        wk_sb.append(wsl)
    for blk in range(4):
        s0 = blk * 512
        psk = qkps.tile([P, 512], f32, name="sc")
        for b2 in range(NB):
            nc.tensor.matmul(
                psk, _pair3(wk_sb[b2], P),
                _pair3(g.hT2[b2], T)[:, :, s0:s0 + 512],
                start=(b2 == 0), stop=(b2 == NB - 1), perf_mode=DR,
            )
        dst = kt[:, s0:s0 + 512]
        if g.has_qb:
            nc.vector.tensor_scalar(
                out=dst, in0=psk, scalar1=g.kb_sb[:, p:p + 1], op0=ALU.add
            )
        else:
            nc.vector.tensor_copy(out=dst, in_=psk)


def _emit_attn_pair(g, p, th, qt, kt, ptp, smp, scps, atps, filler=None):
    """Scores (fp8 operands, transposed), shifted exp to fp8, attn^T via
    DoubleRow + softmax denom (ones column), normalize -> catT2 (fp8).
    `filler()` (if given) is called once per context-tile pair to emit
    PE work that runs while the Act engine chews through the exps."""
    nc = g.nc
    u2, j2 = p // 2, p % 2
    tcols = slice(th * 512, (th + 1) * 512)
    at0 = atps.tile([D + 1, 512], f32, name="ps0")
    at1 = atps.tile([D + 1, 512], f32, name="ps1")
    for u in range(NSU):
        pta = ptp.tile([P, 1024], f8, name="pta")
        ptb = ptp.tile([P, 1024], f8, name="ptb")
        for sj in range(2):
            s = 2 * u + sj
            scols = slice(s * P, (s + 1) * P)
            sc0 = scps.tile([P, 512], f32, name="sc")
            sc1 = scps.tile([P, 512], f32, name="sc")
            # S^T[s,t] = (K^T slice).T @ Q^T slice; the two heads live
            # on row-groups 0-63 / 64-127 so the matmuls pack.
            nc.tensor.matmul(sc0, kt[0:D, scols], qt[0:D, tcols],
                             start=True, stop=True)
            nc.tensor.matmul(sc1, kt[D:2 * D, scols], qt[D:2 * D, tcols],
                             start=True, stop=True)
            nc.scalar.activation(out=pta[:, sj * 512:(sj + 1) * 512],
                                 in_=sc0, func=AF.Exp, bias=g.nshift_t)
            nc.scalar.activation(out=ptb[:, sj * 512:(sj + 1) * 512],
                                 in_=sc1, func=AF.Exp, bias=g.nshift_t)
        if filler is not None:
            filler()
        va_v = _pair3(g.va2[u], H * VP)
        nc.tensor.matmul(
            at0, va_v[:, :, (2 * p) * VP:(2 * p) * VP + VW],
            _pair3(pta, 512),
            start=(u == 0), stop=(u == NSU - 1), perf_mode=DR,
        )
        nc.tensor.matmul(
            at1, va_v[:, :, (2 * p + 1) * VP:(2 * p + 1) * VP + VW],
            _pair3(ptb, 512),
            start=(u == 0), stop=(u == NSU - 1), perf_mode=DR,
        )
    se0 = smp.tile([1, 512], f32, name="se0")
    se1 = smp.tile([1, 512], f32, name="se1")
    dn0 = smp.tile([1, 512], f32, name="dn0")
    dn1 = smp.tile([1, 512], f32, name="dn1")
    # recip_approx_fast misreads PSUM on HW -- bounce rows to SBUF first
    nc.vector.tensor_copy(out=dn0, in_=at0[D:D + 1, :])
    nc.vector.tensor_copy(out=dn1, in_=at1[D:D + 1, :])
    nc.vector.reciprocal_approx_fast(out=se0, in_=dn0)
    nc.vector.reciprocal_approx_fast(out=se1, in_=dn1)
    rb0 = smp.tile([D, 512], f32, name="rb0")
    rb1 = smp.tile([D, 512], f32, name="rb1")
    nc.gpsimd.partition_broadcast(rb0, se0)
    nc.gpsimd.partition_broadcast(rb1, se1)
    c0 = j2 * TQ + th * 512
    nc.vector.tensor_mul(out=g.catT2[u2][0:D, c0:c0 + 512],
                         in0=at0[0:D, :], in1=rb0)
    nc.vector.tensor_mul(out=g.catT2[u2][D:2 * D, c0:c0 + 512],
                         in0=at1[0:D, :], in1=rb1)


def _proj_steps(g, th, xq2p, h2p, pps, t2ps):
    """Per-token-tile proj+LN2 closures (callable inline, in any order)."""
    nc = g.nc

    def step(ts):
        trows = slice(ts * P, (ts + 1) * P)
        xres = xq2p.tile([P, E], bf16, name="xres")
        nc.gpsimd.dma_start(out=xres, in_=g.xkv_d[ts * P:(ts + 1) * P, :])
        psy = [pps.tile([P, 512], f32, name="py") for j in range(2)]
        for u in range(NB):
            lhsT = _pair3(g.catT2[u], TQ)[:, :, trows]
            rhs_t = _pair3(g.pw_sb[u], E)
            for j in range(2):
                nc.tensor.matmul(
                    psy[j], lhsT, rhs_t[:, :, j * 512:(j + 1) * 512],
                    start=(u == 0), stop=(u == NB - 1), perf_mode=DR,
                )
        x2 = xq2p.tile([P, E], bf16, name="x2t")
        for j in range(2):
            jc = slice(j * 512, (j + 1) * 512)
            if g.has_pb:
                nc.vector.tensor_add(out=x2[:, jc], in0=psy[j], in1=g.pb_bc[:, jc])
                nc.vector.tensor_add(out=x2[:, jc], in0=x2[:, jc],
                                     in1=xres[:, jc])
            else:
                nc.vector.tensor_add(out=x2[:, jc], in0=psy[j],
                                     in1=xres[:, jc])
        nc.sync.dma_start(out=g.x2_d[ts * P:(ts + 1) * P, :], in_=x2)
        h2 = h2p.tile([P, E], bf16, name="h2")
        _emit_ln(g, x2, h2)
        for c in range(NE):
            tp = t2ps.tile([P, P], bf16, name="py")
            nc.tensor.transpose(tp, h2[:, c * P:(c + 1) * P], g.ident)
            nc.scalar.copy(out=g.h2T[c][:, trows], in_=tp)

    return [(lambda ts=ts: step(ts)) for ts in range(th * 4, th * 4 + 4)]


TQQ = 256  # token quarter


def _mlp_steps(g, q, pup, dnp, dwps, outp):
    """Closure list for quarter q: 32 up steps, then per e-half 32 down
    steps + a finish. Each step is ~0.5-1.7us of PE work, emitted as
    attention filler so it runs under the Act engine's exp stream."""
    nc = g.nc
    qcols = slice(q * TQQ, (q + 1) * TQQ)
    state = {}

    def up_step(f):
        pu = pup.tile([P, TQQ], f32, name="pu")
        for c in range(NE):
            nc.tensor.matmul(
                pu, g.uw_sb[c][:, f * P:(f + 1) * P], g.h2T[c][:, qcols],
                start=(c == 0), stop=(c == NE - 1),
            )
        nc.scalar.activation(out=g.hid_tiles[f], in_=pu, func=AF.Relu,
                             bias=g.ub_sb[:, f:f + 1])

    def down_step(j, f):
        jc = slice(j * 512, (j + 1) * 512)
        if f == 0:
            state["dn"] = [dnp.tile([P, 512], f32, name=f"dnq{t2}")
                           for t2 in range(2)]
        dn = state["dn"]
        dwt = dwps.tile([P, 512], bf16, name="dwt")
        nc.sync.dma_start(out=dwt, in_=g.dw_d[f][:, jc])
        for t2 in range(2):
            nc.tensor.matmul(
                dn[t2], g.hid_tiles[f][:, t2 * P:(t2 + 1) * P], dwt,
                start=(f == 0), stop=(f == NF - 1),
            )

    def finish(j):
        jc = slice(j * 512, (j + 1) * 512)
        dn = state["dn"]
        for t2 in range(2):
            ti = q * 2 + t2
            x2s = outp.tile([P, 512], bf16, name="x2s")
            nc.gpsimd.dma_start(out=x2s, in_=g.x2_d[ti * P:(ti + 1) * P, jc])
            ot = outp.tile([P, 512], f32, name="ot")
            if g.has_db:
                nc.vector.tensor_add(out=ot, in0=dn[t2], in1=g.db_bc[:, jc])
                nc.vector.tensor_add(out=ot, in0=ot, in1=x2s)
            else:
                nc.vector.tensor_add(out=ot, in0=dn[t2], in1=x2s)
            nc.sync.dma_start(out=g.out_d[ti * P:(ti + 1) * P, jc], in_=ot)

    steps = [(lambda f=f: up_step(f)) for f in range(NF)]
    for j in range(2):
        steps += [(lambda j=j, f=f: down_step(j, f)) for f in range(NF)]
        steps.append(lambda j=j: finish(j))
    return steps


def _emit_mlp_quarter_wide(g, q, hidp, dwps, outp, upps, dnps):
    """Tail variant: f-outer down with full-width dw loads and 2x[P,E] dn."""
    nc = g.nc
    qcols = slice(q * TQQ, (q + 1) * TQQ)
    dn = [dnps.tile([P, E], f32, name=f"dnw{j}") for j in range(2)]
    for f in range(NF):
        pu = upps.tile([P, TQQ], f32, name="py")
        for c in range(NE):
            nc.tensor.matmul(
                pu, g.uw_sb[c][:, f * P:(f + 1) * P], g.h2T[c][:, qcols],
                start=(c == 0), stop=(c == NE - 1),
            )
        hid = g.hid_tiles[f]
        nc.scalar.activation(out=hid, in_=pu, func=AF.Relu,
                             bias=g.ub_sb[:, f:f + 1])
        dwt = dwps.tile([P, E], bf16, name="dwf")
        nc.sync.dma_start(out=dwt, in_=g.dw_d[f])
        for t2 in range(2):
            for j in range(2):
                nc.tensor.matmul(
                    dn[t2][:, j * 512:(j + 1) * 512],
                    hid[:, t2 * P:(t2 + 1) * P],
                    dwt[:, j * 512:(j + 1) * 512],
                    start=(f == 0), stop=(f == NF - 1),
                )
    for t2 in range(2):
        ti = q * 2 + t2
        x2s = outp.tile([P, E], bf16, name="x2w")
        nc.gpsimd.dma_start(out=x2s, in_=g.x2_d[ti * P:(ti + 1) * P, :])
        ot = outp.tile([P, E], f32, name="otw")
        if g.has_db:
            nc.vector.tensor_add(out=ot, in0=dn[t2], in1=g.db_bc)
            nc.vector.tensor_add(out=ot, in0=ot, in1=x2s)
        else:
            nc.vector.tensor_add(out=ot, in0=dn[t2], in1=x2s)
        nc.sync.dma_start(out=g.out_d[ti * P:(ti + 1) * P, :], in_=ot)





def _build(flags, reps=1):
    has_qb, has_pb, has_db = flags
    nc = bacc.Bacc("TRN2", target_bir_lowering=False, debug=False, num_devices=8)

    g = _Ctx()
    g.nc = nc
    g.has_qb, g.has_pb, g.has_db = flags
    g.xkv_d = nc.dram_tensor("xkv", [T, E], f32, kind="ExternalInput")
    g.wq_d = nc.dram_tensor("wq", [NB, NPAIR, P, 2 * P], f8, kind="ExternalInput")
    g.wk_d = nc.dram_tensor("wk", [NB, NPAIR, P, 2 * P], f8, kind="ExternalInput")
    g.wv_d = nc.dram_tensor("wv", [NB, P, 2 * E], f8, kind="ExternalInput")
    g.vrow_d = nc.dram_tensor("vrow", [1, 2 * H * VP], f8, kind="ExternalInput")
    g.pw_d = nc.dram_tensor("pw", [NB, P, 2 * E], f8, kind="ExternalInput")
    g.uw_d = nc.dram_tensor("uw", [NE, P, F], bf16, kind="ExternalInput")
    g.ub_d = nc.dram_tensor("ub", [P, NF], f32, kind="ExternalInput")
    g.dw_d = nc.dram_tensor("dw", [NF, P, E], bf16, kind="ExternalInput")
    if has_qb:
        g.qb_d = nc.dram_tensor("qb", [P, NPAIR], f32, kind="ExternalInput")
        g.kb_d = nc.dram_tensor("kb", [P, NPAIR], f32, kind="ExternalInput")
        g.vbrow_d = nc.dram_tensor("vbrow", [1, E], bf16, kind="ExternalInput")
    if has_pb:
        g.pbrow_d = nc.dram_tensor("pbrow", [1, E], f32, kind="ExternalInput")
    if has_db:
        g.dbrow_d = nc.dram_tensor("dbrow", [1, E], f32, kind="ExternalInput")
    g.x2_d = nc.dram_tensor("x2s", [TQ, E], bf16, kind="Internal")
    g.out_d = nc.dram_tensor("out", [TQ, E], f32, kind="ExternalOutput")

    with tile.TileContext(nc) as tc:
        with (
            tc.tile_pool(name="consts", bufs=1) as consts,
            tc.tile_pool(name="stat", bufs=4) as stat,
            tc.tile_pool(name="catp", bufs=1) as catp,
            tc.tile_pool(name="x2p", bufs=1) as x2p,
            tc.tile_pool(name="h2Tp", bufs=1) as h2Tp,
        ):
            g.consts, g.stat = consts, stat
            _emit_consts(g)
            for _rep in range(reps):
                _emit_all(g, tc, catp, x2p, h2Tp)

    nc.finalize()
    return nc


def _emit_all(g, tc, catp, x2p, h2Tp):
    g.catT2 = [catp.tile([P, 2 * TQ], f8, name=f"catT{u}") for u in range(NB)]
    g.h2T = [h2Tp.tile([P, TQ], bf16, name=f"h2T{c}") for c in range(NE)]

    g.pwp = tc.alloc_tile_pool(name="pwp", bufs=1)
    g.uwp = tc.alloc_tile_pool(name="uwp", bufs=1)
    hidp = tc.alloc_tile_pool(name="hidp", bufs=1)
    g.hid_tiles = [hidp.tile([P, TQQ], bf16, name=f"hid{f}")
                   for f in range(NF)]

    with tc.tile_pool(name="vaug", bufs=1) as vap:
        g.va2 = [vap.tile([P, 2 * H * VP], f8, name=f"va{u}")
                 for u in range(NSU)]
        qktp = tc.alloc_tile_pool(name="qktp", bufs=1)
        qts = [qktp.tile([P, TQ], f8, name=f"qt{p}") for p in range(NPAIR)]
        kts = [qktp.tile([P, T], f8, name=f"kt{p}") for p in range(NPAIR)]

        with tc.tile_pool(name="hTp", bufs=1) as hTp:
            g.hT2 = [hTp.tile([P, 2 * T], f8, name=f"hT{b}") for b in range(NB)]
            with (
                tc.tile_pool(name="hp", bufs=4) as hp,
                tc.tile_pool(name="xk", bufs=3) as xkp,
                tc.tile_pool(name="tps", bufs=4, space="PSUM") as tps,
                tc.tile_pool(name="wvp", bufs=1) as wvp,
                tc.tile_pool(name="vps", bufs=2, space="PSUM") as vps,
            ):
                g.hp = hp
                _emit_ln1_v(g, xkp, tps, wvp, vps)

            g.pw_sb = []
            for u in range(NB):
                w = g.pwp.tile([P, 2 * E], f8, name=f"pw{u}")
                g.nc.gpsimd.dma_start(out=w, in_=g.pw_d[u])
                g.pw_sb.append(w)

            # th=0 attention; Q/K for pair p+1 emitted as PE filler under
            # pair p's exp stream (pair 0's Q/K runs upfront)
            with (
                tc.tile_pool(name="ptp0", bufs=3) as ptp0,
                tc.tile_pool(name="smp0", bufs=1) as smp0,
                tc.tile_pool(name="wqk", bufs=6) as wqkp,
                tc.tile_pool(name="atps", bufs=2, space="PSUM") as atps,
                tc.tile_pool(name="scps", bufs=4, space="PSUM") as scps,
            ):
                for st in _qk_steps(g, 0, qts[0], kts[0], wqkp, scps):
                    st()
                qk_queue = []
                qi = {"i": 0}

                def qk_filler(n=1):
                    for _ in range(n):
                        if qi["i"] < len(qk_queue):
                            qk_queue[qi["i"]]()
                            qi["i"] += 1

                for p in range(NPAIR):
                    if p + 1 < NPAIR:
                        qk_queue.extend(
                            _qk_steps(g, p + 1, qts[p + 1], kts[p + 1],
                                      wqkp, scps))
                    _emit_attn_pair(g, p, 0, qts[p], kts[p], ptp0, smp0,
                                    scps, atps, filler=qk_filler)
                qk_filler(len(qk_queue))

        # hT2 freed; proj th0, then th1 attention with MLP q0/q1 as filler
        with (
            tc.tile_pool(name="ptp", bufs=3) as ptp,
            tc.tile_pool(name="smp", bufs=1) as smp,
            tc.tile_pool(name="xq2", bufs=2) as xq2p,
            tc.tile_pool(name="h2p", bufs=2) as h2p,
            tc.tile_pool(name="dwps", bufs=4) as dwps,
            tc.tile_pool(name="outp", bufs=2) as outp,
        ):
            g.uw_sb = []
            for c in range(NE):
                w = g.uwp.tile([P, F], bf16, name=f"uw{c}")
                g.nc.gpsimd.dma_start(out=w, in_=g.uw_d[c])
                g.uw_sb.append(w)
            with (
                tc.tile_pool(name="pps", bufs=2, space="PSUM") as pps,
                tc.tile_pool(name="t2ps", bufs=2, space="PSUM") as t2ps,
            ):
                for st in _proj_steps(g, 0, xq2p, h2p, pps, t2ps):
                    st()
            with (
                tc.tile_pool(name="atps2", bufs=1, space="PSUM") as atps2,
                tc.tile_pool(name="scps2", bufs=2, space="PSUM") as scps2,
                tc.tile_pool(name="pup", bufs=2, space="PSUM") as pup,
                tc.tile_pool(name="dnp", bufs=1, space="PSUM") as dnp,
            ):
                steps = (_mlp_steps(g, 0, pup, dnp, dwps, outp)
                         + _mlp_steps(g, 1, pup, dnp, dwps, outp))
                si = {"i": 0}

                def filler(n=3):
                    for _ in range(n):
                        if si["i"] < len(steps):
                            steps[si["i"]]()
                            si["i"] += 1

                for p in range(NPAIR):
                    _emit_attn_pair(g, p, 1, qts[p], kts[p], ptp, smp,
                                    scps2, atps2, filler=filler)
                filler(len(steps))  # flush leftovers
        qktp.release()

    # proj-th1 interleaved with the MLP tail: quarter q2 needs only the
    # first two proj tiles (ts4/ts5), so it runs on the PE while ts6/ts7's
    # LN2 chains occupy DVE/Act; q3 follows ts7.
    with (
        tc.tile_pool(name="xq2b", bufs=2) as xq2b,
        tc.tile_pool(name="h2pb", bufs=2) as h2pb,
        tc.tile_pool(name="dwpsT", bufs=4) as dwpsT,
        tc.tile_pool(name="outpT", bufs=2) as outpT,
        tc.tile_pool(name="ppsB", bufs=2, space="PSUM") as ppsB,
        tc.tile_pool(name="dnpsT", bufs=1, space="PSUM") as dnpsT,
    ):
        psteps = _proj_steps(g, 1, xq2b, h2pb, ppsB, ppsB)
        psteps[0]()
        psteps[1]()
        _emit_mlp_quarter_wide(g, 2, hidp, dwpsT, outpT, ppsB, dnpsT)
        psteps[2]()
        psteps[3]()
        _emit_mlp_quarter_wide(g, 3, hidp, dwpsT, outpT, ppsB, dnpsT)
    hidp.release()
    g.uwp.release()
    g.pwp.release()


def _get_nc(flags, reps=1):
    key = (flags, reps)
    if key not in _BUILD_CACHE:
        _BUILD_CACHE[key] = _build(flags, reps)
    return _BUILD_CACHE[key]


def _q8(x):
    return np.clip(np.asarray(x, np.float32), -240.0, 240.0).astype(
        ml_dtypes.float8_e4m3)


def _prep(x, Wq, Wk, Wv, proj_w, proj_b, ln1_g, ln1_b, ln2_g, ln2_b,
          up_w, up_b, down_w, down_b):
    """Host-side shard + weight fold/cast/layout. Returns (flags, in_maps)."""
    bfl = ml_dtypes.bfloat16
    x = np.ascontiguousarray(np.asarray(x, dtype=np.float32))
    Wq = np.asarray(Wq, np.float32)
    Wk = np.asarray(Wk, np.float32)
    Wv = np.asarray(Wv, np.float32)
    g1 = np.asarray(ln1_g, np.float32)
    b1 = np.asarray(ln1_b, np.float32)
    g2 = np.asarray(ln2_g, np.float32)
    b2 = np.asarray(ln2_b, np.float32)
    proj_w = np.asarray(proj_w, np.float32)
    up_w = np.asarray(up_w, np.float32)
    down_w = np.asarray(down_w, np.float32)

    # [H, E, D] -> [E, H*D]; fold attention scale into Q, LN1 gain into all
    wq_all = (Wq * (D ** -0.5)).transpose(1, 0, 2).reshape(E, E)
    wk_all = Wk.transpose(1, 0, 2).reshape(E, E)
    wv_all = Wv.transpose(1, 0, 2).reshape(E, E)
    qb_vec = b1 @ wq_all
    kb_vec = b1 @ wk_all
    vb_vec = b1 @ wv_all
    wq_f = g1[:, None] * wq_all
    wk_f = g1[:, None] * wk_all
    wv_f = g1[:, None] * wv_all

    def _pair_chunks_qk(w):  # [E, E] -> [NB, NPAIR, P, 2P], DR pair layout
        return np.ascontiguousarray(
            _q8(w.reshape(NB, 2, P, NPAIR, P).transpose(0, 3, 2, 1, 4)
                .reshape(NB, NPAIR, P, 2 * P)))

    def _pair_rows(w, ncols):  # [E_in, ncols] -> [E_in/256, P, 2*ncols]
        nb = w.shape[0] // 256
        return np.ascontiguousarray(
            _q8(w.reshape(nb, 2, P, ncols).transpose(0, 2, 1, 3)
                .reshape(nb, P, 2 * ncols)))

    vrow = np.zeros((1, 2 * H * VP), np.float32)
    vrow.reshape(2, H, VP)[:, :, D] = 1.0

    uw_f = g2[:, None] * up_w
    ub_f = np.asarray(up_b, np.float32) + b2 @ up_w

    has_qb = bool(np.any(b1 != 0))
    has_pb = bool(np.any(np.asarray(proj_b) != 0))
    has_db = bool(np.any(np.asarray(down_b) != 0))
    flags = (has_qb, has_pb, has_db)

    shared = {
        "wq": _pair_chunks_qk(wq_f),
        "wk": _pair_chunks_qk(wk_f),
        "wv": _pair_rows(wv_f, E),
        "vrow": _q8(vrow),
        "pw": _pair_rows(proj_w, E),
        "uw": np.ascontiguousarray(uw_f.reshape(NE, P, F).astype(bfl)),
        "ub": np.ascontiguousarray(ub_f.reshape(NF, P).T.astype(np.float32)),
        "dw": np.ascontiguousarray(down_w.reshape(NF, P, E).astype(bfl)),
    }
    if has_qb:
        shared["qb"] = np.ascontiguousarray(
            qb_vec.reshape(NPAIR, P).T.astype(np.float32))
        shared["kb"] = np.ascontiguousarray(
            kb_vec.reshape(NPAIR, P).T.astype(np.float32))
        shared["vbrow"] = vb_vec.reshape(1, E).astype(bfl)
    if has_pb:
        shared["pbrow"] = np.asarray(proj_b, np.float32).reshape(1, E)
    if has_db:
        shared["dbrow"] = np.asarray(down_b, np.float32).reshape(1, E)

    in_maps = []
    for c in range(8):
        b, half = c // 2, c % 2
        xb = x[b]
        if half == 1:
            xb = np.concatenate([xb[TQ:], xb[:TQ]], axis=0)
        in_maps.append({"xkv": np.ascontiguousarray(xb), **shared})
    return flags, in_maps


def kernel(**inputs) -> np.ndarray:
    flags, in_maps = _prep(**inputs)
    nc = _get_nc(flags)
    res = run_bass_kernel_spmd(nc, in_maps, core_ids=list(range(8)))
    out = np.empty((B, T, E), np.float32)
    for c in range(8):
        b, half = c // 2, c % 2
        out[b, half * TQ:(half + 1) * TQ, :] = res.results[c]["out"]
    return out


# revision 8
# speedup vs baseline: 1.0626x; 1.0475x over previous
"""Trainium2 Bass kernel for a dense transformer block (nn_Block_120259084502).

fp8 DoubleRow variant: attention matmuls (Q/K/V projections, attn@V, output
proj) run in fp8-e4m3 with DoubleRow perf mode (2 contraction elements per
cycle -> half the matmul streams). Scores (K=64/head, no DR gain) and the
MLP (fp8 fails the error budget there) stay bf16. Softmax runs shifted
(exp(S-2)) so probabilities fit fp8-e4m3's +/-240 range; the shift cancels
in normalization. LayerNorm statistics and both residuals stay fp32.

Sharding: core c handles batch c//2, query-token half c%2. Each core gets
its batch's full 2048 tokens (rolled so its own 1024 query tokens come
first) and computes K/V for all of them locally; no collectives.
"""

import numpy as np
import ml_dtypes

import concourse.bacc as bacc
import concourse.tile as tile
from concourse import mybir
from concourse.bass_utils import run_bass_kernel_spmd
from concourse.masks import make_identity

bf16 = mybir.dt.bfloat16
f8 = mybir.dt.float8e4
f32 = mybir.dt.float32
AF = mybir.ActivationFunctionType
ALU = mybir.AluOpType
DR = mybir.MatmulPerfMode.DoubleRow

P = 128
B, T, E, H, D = 4, 2048, 1024, 16, 64
F = 4 * E                    # 4096 MLP hidden
TQ = T // 2                  # 1024 own query tokens per core
NE = E // P                  # 8 e-chunks
NB = NE // 2                 # 4 e-chunk pairs (DoubleRow K blocks)
NPAIR = H // 2               # 8 head pairs
NST = T // P                 # 16 context-token tiles
NSU = NST // 2               # 8 context-token tile pairs
NTS = TQ // P                # 8 own-token tiles
NF = F // P                  # 32 f-chunks
VW = D + 1                   # per-head V width incl. ones column
VP = 80                      # padded V width (16B-aligned fp8 LDW slices)
LN_EPS = 1e-5
SHIFT = 3.0                  # softmax exp shift (cancels in normalization)

_BUILD_CACHE = {}


class _Ctx:
    """Shared build state passed between phase emitters."""
    pass


def _pair3(t, inner):
    """[P, 2*inner_total] tile -> [P, 2, inner_total] AP."""
    return t.rearrange("p (j w) -> p j w", j=2)


def _emit_ln(g, xt, out_bf):
    nc = g.nc
    st = g.stat.tile([P, 2, nc.vector.BN_STATS_DIM], f32, name="bnst")
    xv = xt.rearrange("p (s g) -> p s g", s=2)
    nc.vector.bn_stats(out=st[:, 0, :], in_=xv[:, 0, :])
    nc.vector.bn_stats(out=st[:, 1, :], in_=xv[:, 1, :])
    mv = g.stat.tile([P, nc.vector.BN_AGGR_DIM], f32, name="bnmv")
    nc.vector.bn_aggr(out=mv, in_=st)
    rstd = g.stat.tile([P, 1], f32, name="bnrs")
    nc.scalar.activation(out=rstd, in_=mv[:, 1:2], func=AF.Sqrt, bias=g.eps_t)
    nc.vector.reciprocal(out=rstd, in_=rstd)
    nc.vector.tensor_scalar(
        out=out_bf, in0=xt, scalar1=mv[:, 0:1], scalar2=rstd,
        op0=ALU.subtract, op1=ALU.mult,
    )


def _emit_consts(g):
    nc, consts = g.nc, g.consts
    g.ident = consts.tile([P, P], bf16, name="ident")
    make_identity(nc, g.ident)
    g.eps_t = consts.tile([P, 1], f32, name="eps")
    nc.vector.memset(g.eps_t, LN_EPS)
    g.nshift_t = consts.tile([P, 1], f32, name="nshift")
    nc.vector.memset(g.nshift_t, -SHIFT)
    g.ub_sb = consts.tile([P, NF], f32, name="ubsb")
    nc.sync.dma_start(out=g.ub_sb, in_=g.ub_d[:, :])
    if g.has_qb:
        g.qb_sb = consts.tile([P, NPAIR], f32, name="qbsb")
        nc.sync.dma_start(out=g.qb_sb, in_=g.qb_d[:, :])
        g.kb_sb = consts.tile([P, NPAIR], f32, name="kbsb")
        nc.sync.dma_start(out=g.kb_sb, in_=g.kb_d[:, :])
        g.vb_bc = consts.tile([P, E], bf16, name="vbbc")
        nc.gpsimd.dma_start(
            out=g.vb_bc, in_=g.vbrow_d.ap()[0:1, :].partition_broadcast(P)[:, 0, :]
        )
    if g.has_pb:
        g.pb_bc = consts.tile([P, E], f32, name="pbbc")
        nc.gpsimd.dma_start(
            out=g.pb_bc, in_=g.pbrow_d.ap()[0:1, :].partition_broadcast(P)[:, 0, :]
        )
    if g.has_db:
        g.db_bc = consts.tile([P, E], f32, name="dbbc")
        nc.gpsimd.dma_start(
            out=g.db_bc, in_=g.dbrow_d.ap()[0:1, :].partition_broadcast(P)[:, 0, :]
        )


def _emit_ln1_v(g, xkp, tps, wvp, vps):
    """LN1 + transpose + V projection, software-pipelined: V for token tile
    i-3 is emitted after LN tile i, so the PE's V matmuls never wait on the
    just-issued hT2 copies. hT2 copies alternate Scalar/Vector."""
    nc = g.nc
    LAG = 3
    wv_sb = []
    for b2 in range(NB):
        w = wvp.tile([P, 2 * E], f8, name=f"wv{b2}")
        nc.sync.dma_start(out=w, in_=g.wv_d[b2])
        wv_sb.append(w)
    for u in range(NSU):
        nc.gpsimd.dma_start(
            out=g.va2[u],
            in_=g.vrow_d.ap()[0:1, :].partition_broadcast(P)[:, 0, :],
        )

    def emit_v(s):
        u, sj = s // 2, s % 2
        pv = [vps.tile([P, 512], f32, name="pv") for _ in range(2)]
        scols = slice(s * P, (s + 1) * P)
        for b2 in range(NB):
            lhsT = _pair3(g.hT2[b2], T)[:, :, scols]
            rhs_t = _pair3(wv_sb[b2], E)
            for j in range(2):
                nc.tensor.matmul(
                    pv[j], lhsT, rhs_t[:, :, j * 512:(j + 1) * 512],
                    start=(b2 == 0), stop=(b2 == NB - 1), perf_mode=DR,
                )
        va_v = g.va2[u].rearrange("p (j h c) -> p j h c", j=2, c=VP)
        for j in range(2):
            dst = va_v[:, sj, j * 8:(j + 1) * 8, 0:D]
            src = pv[j].rearrange("p (h d) -> p h d", d=D)
            if g.has_qb:
                vb_view = g.vb_bc.rearrange("p (h d) -> p h d", d=D)[
                    :, j * 8:(j + 1) * 8, :
                ]
                nc.vector.tensor_add(out=dst, in0=src, in1=vb_view)
            else:
                nc.scalar.copy(out=dst, in_=src)

    for i in range(NST):
        xt = xkp.tile([P, E], f32, name="xk")
        nc.sync.dma_start(out=xt, in_=g.xkv_d[i * P:(i + 1) * P, :])
        ht = g.hp.tile([P, E], bf16, name="h")
        _emit_ln(g, xt, ht)
        for c in range(NE):
            tp = tps.tile([P, P], bf16, name="tp")
            nc.tensor.transpose(tp, ht[:, c * P:(c + 1) * P], g.ident)
            dst = g.hT2[c // 2][:, (c % 2) * T + i * P:(c % 2) * T + (i + 1) * P]
            if c % 2 == 0:
                nc.scalar.copy(out=dst, in_=tp)
            else:
                nc.vector.tensor_copy(out=dst, in_=tp)
        if i >= LAG:
            emit_v(i - LAG)
    for s in range(NST - LAG, NST):
        emit_v(s)


def _qk_steps(g, p, qt, kt, wqkp, qkps):
    """Step closures for pair p's Q/K projections (filler for th0 attn)."""
    nc = g.nc
    state = {}

    def s_load():
        state["wq"], state["wk"] = [], []
        for b2 in range(NB):
            wsl = wqkp.tile([P, 2 * P], f8, name="wsl")
            nc.sync.dma_start(out=wsl, in_=g.wq_d[b2, p])
            state["wq"].append(wsl)
        for b2 in range(NB):
            wsl = wqkp.tile([P, 2 * P], f8, name="wsl")
            nc.sync.dma_start(out=wsl, in_=g.wk_d[b2, p])
            state["wk"].append(wsl)

    def s_q(j):
        psq = qkps.tile([P, 512], f32, name="sc")
        for b2 in range(NB):
            nc.tensor.matmul(
                psq, _pair3(state["wq"][b2], P),
                _pair3(g.hT2[b2], T)[:, :, j * 512:(j + 1) * 512],
                start=(b2 == 0), stop=(b2 == NB - 1), perf_mode=DR,
            )
        dst = qt[:, j * 512:(j + 1) * 512]
        if g.has_qb:
            nc.vector.tensor_scalar(
                out=dst, in0=psq, scalar1=g.qb_sb[:, p:p + 1], op0=ALU.add
            )
        else:
            nc.vector.tensor_copy(out=dst, in_=psq)

    def s_k(blk):
        s0 = blk * 512
        psk = qkps.tile([P, 512], f32, name="sc")
        for b2 in range(NB):
            nc.tensor.matmul(
                psk, _pair3(state["wk"][b2], P),
                _pair3(g.hT2[b2], T)[:, :, s0:s0 + 512],
                start=(b2 == 0), stop=(b2 == NB - 1), perf_mode=DR,
            )
        dst = kt[:, s0:s0 + 512]
        if g.has_qb:
            nc.vector.tensor_scalar(
                out=dst, in0=psk, scalar1=g.kb_sb[:, p:p + 1], op0=ALU.add
            )
        else:
            nc.vector.tensor_copy(out=dst, in_=psk)

    return ([s_load] + [(lambda j=j: s_q(j)) for j in range(2)]
            + [(lambda b=b: s_k(b)) for b in range(4)])


def _emit_qkt_pair(g, p, qt, kt, wqkp, qkps):
    """Q^T and K^T for head pair p: [128 (2 heads x 64d), tokens], bf16.
    fp8 DoubleRow over e-chunk pairs."""
    nc = g.nc
    wq_sb = []
    for b2 in range(NB):
        wsl = wqkp.tile([P, 2 * P], f8, name="wsl")
        nc.sync.dma_start(out=wsl, in_=g.wq_d[b2, p])
        wq_sb.append(wsl)
    for j in range(2):
        psq = qkps.tile([P, 512], f32, name="sc")
        for b2 in range(NB):
            nc.tensor.matmul(
                psq, _pair3(wq_sb[b2], P),
                _pair3(g.hT2[b2], T)[:, :, j * 512:(j + 1) * 512],
                start=(b2 == 0), stop=(b2 == NB - 1), perf_mode=DR,
            )
        dst = qt[:, j * 512:(j + 1) * 512]
        if g.has_qb:
            nc.vector.tensor_scalar(
                out=dst, in0=psq, scalar1=g.qb_sb[:, p:p + 1], op0=ALU.add
            )
        else:
            nc.vector.tensor_copy(out=dst, in_=psq)
    wk_sb = []
    for b2 in range(NB):
        wsl = wqkp.tile([P, 2 * P], f8, name="wsl")
        nc.sync.dma_start(out=wsl, in_=g.wk_d[b2, p])
        wk_sb.append(wsl)
    for blk in range(4):
        s0 = blk * 512
        psk = qkps.tile([P, 512], f32, name="sc")
        for b2 in range(NB):
            nc.tensor.matmul(
                psk, _pair3(wk_sb[b2], P),
                _pair3(g.hT2[b2], T)[:, :, s0:s0 + 512],
                start=(b2 == 0), stop=(b2 == NB - 1), perf_mode=DR,
            )
        dst = kt[:, s0:s0 + 512]
        if g.has_qb:
            nc.vector.tensor_scalar(
                out=dst, in0=psk, scalar1=g.kb_sb[:, p:p + 1], op0=ALU.add
            )
        else:
            nc.vector.tensor_copy(out=dst, in_=psk)


def _emit_attn_pair(g, p, th, qt, kt, ptp, smp, scps, atps, filler=None):
    """Scores (fp8 operands, transposed), shifted exp to fp8, attn^T via
    DoubleRow + softmax denom (ones column), normalize -> catT2 (fp8).
    `filler()` (if given) is called once per context-tile pair to emit
    PE work that runs while the Act engine chews through the exps."""
    nc = g.nc
    u2, j2 = p // 2, p % 2
    tcols = slice(th * 512, (th + 1) * 512)
    at0 = atps.tile([D + 1, 512], f32, name="ps0")
    at1 = atps.tile([D + 1, 512], f32, name="ps1")
    for u in range(NSU):
        pta = ptp.tile([P, 1024], f8, name="pta")
        ptb = ptp.tile([P, 1024], f8, name="ptb")
        for sj in range(2):
            s = 2 * u + sj
            scols = slice(s * P, (s + 1) * P)
            sc0 = scps.tile([P, 512], f32, name="sc")
            sc1 = scps.tile([P, 512], f32, name="sc")
            # S^T[s,t] = (K^T slice).T @ Q^T slice; the two heads live
            # on row-groups 0-63 / 64-127 so the matmuls pack.
            nc.tensor.matmul(sc0, kt[0:D, scols], qt[0:D, tcols],
                             start=True, stop=True)
            nc.tensor.matmul(sc1, kt[D:2 * D, scols], qt[D:2 * D, tcols],
                             start=True, stop=True)
            nc.scalar.activation(out=pta[:, sj * 512:(sj + 1) * 512],
                                 in_=sc0, func=AF.Exp, bias=g.nshift_t)
            nc.scalar.activation(out=ptb[:, sj * 512:(sj + 1) * 512],
                                 in_=sc1, func=AF.Exp, bias=g.nshift_t)
        if filler is not None:
            filler()
        va_v = _pair3(g.va2[u], H * VP)
        nc.tensor.matmul(
            at0, va_v[:, :, (2 * p) * VP:(2 * p) * VP + VW],
            _pair3(pta, 512),
            start=(u == 0), stop=(u == NSU - 1), perf_mode=DR,
        )
        nc.tensor.matmul(
            at1, va_v[:, :, (2 * p + 1) * VP:(2 * p + 1) * VP + VW],
            _pair3(ptb, 512),
            start=(u == 0), stop=(u == NSU - 1), perf_mode=DR,
        )
    se0 = smp.tile([1, 512], f32, name="se0")
    se1 = smp.tile([1, 512], f32, name="se1")
    dn0 = smp.tile([1, 512], f32, name="dn0")
    dn1 = smp.tile([1, 512], f32, name="dn1")
    # recip_approx_fast misreads PSUM on HW -- bounce rows to SBUF first
    nc.vector.tensor_copy(out=dn0, in_=at0[D:D + 1, :])
    nc.vector.tensor_copy(out=dn1, in_=at1[D:D + 1, :])
    nc.vector.reciprocal_approx_fast(out=se0, in_=dn0)
    nc.vector.reciprocal_approx_fast(out=se1, in_=dn1)
    rb0 = smp.tile([D, 512], f32, name="rb0")
    rb1 = smp.tile([D, 512], f32, name="rb1")
    nc.gpsimd.partition_broadcast(rb0, se0)
    nc.gpsimd.partition_broadcast(rb1, se1)
    c0 = j2 * TQ + th * 512
    nc.vector.tensor_mul(out=g.catT2[u2][0:D, c0:c0 + 512],
                         in0=at0[0:D, :], in1=rb0)
    nc.vector.tensor_mul(out=g.catT2[u2][D:2 * D, c0:c0 + 512],
                         in0=at1[0:D, :], in1=rb1)


def _proj_steps(g, th, xq2p, h2p, pps, t2ps):
    """Per-token-tile proj+LN2 closures (callable inline, in any order)."""
    nc = g.nc

    def step(ts):
        trows = slice(ts * P, (ts + 1) * P)
        xres = xq2p.tile([P, E], bf16, name="xres")
        nc.gpsimd.dma_start(out=xres, in_=g.xkv_d[ts * P:(ts + 1) * P, :])
        psy = [pps.tile([P, 512], f32, name="py") for j in range(2)]
        for u in range(NB):
            lhsT = _pair3(g.catT2[u], TQ)[:, :, trows]
            rhs_t = _pair3(g.pw_sb[u], E)
            for j in range(2):
                nc.tensor.matmul(
                    psy[j], lhsT, rhs_t[:, :, j * 512:(j + 1) * 512],
                    start=(u == 0), stop=(u == NB - 1), perf_mode=DR,
                )
        x2 = xq2p.tile([P, E], bf16, name="x2t")
        for j in range(2):
            jc = slice(j * 512, (j + 1) * 512)
            if g.has_pb:
                nc.vector.tensor_add(out=x2[:, jc], in0=psy[j], in1=g.pb_bc[:, jc])
                nc.vector.tensor_add(out=x2[:, jc], in0=x2[:, jc],
                                     in1=xres[:, jc])
            else:
                nc.vector.tensor_add(out=x2[:, jc], in0=psy[j],
                                     in1=xres[:, jc])
        nc.sync.dma_start(out=g.x2_d[ts * P:(ts + 1) * P, :], in_=x2)
        h2 = h2p.tile([P, E], bf16, name="h2")
        _emit_ln(g, x2, h2)
        for c in range(NE):
            tp = t2ps.tile([P, P], bf16, name="py")
            nc.tensor.transpose(tp, h2[:, c * P:(c + 1) * P], g.ident)
            nc.scalar.copy(out=g.h2T[c][:, trows], in_=tp)

    return [(lambda ts=ts: step(ts)) for ts in range(th * 4, th * 4 + 4)]


TQQ = 256  # token quarter


def _mlp_steps(g, q, pup, dnp, dwps, outp):
    """Closure list for quarter q: 32 up steps, then per e-half 32 down
    steps + a finish. Each step is ~0.5-1.7us of PE work, emitted as
    attention filler so it runs under the Act engine's exp stream."""
    nc = g.nc
    qcols = slice(q * TQQ, (q + 1) * TQQ)
    state = {}

    def up_step(f):
        pu = pup.tile([P, TQQ], f32, name="pu")
        for c in range(NE):
            nc.tensor.matmul(
                pu, g.uw_sb[c][:, f * P:(f + 1) * P], g.h2T[c][:, qcols],
                start=(c == 0), stop=(c == NE - 1),
            )
        nc.scalar.activation(out=g.hid_tiles[f], in_=pu, func=AF.Relu,
                             bias=g.ub_sb[:, f:f + 1])

    def down_step(j, f):
        jc = slice(j * 512, (j + 1) * 512)
        if f == 0:
            state["dn"] = [dnp.tile([P, 512], f32, name=f"dnq{t2}")
                           for t2 in range(2)]
        dn = state["dn"]
        dwt = dwps.tile([P, 512], bf16, name="dwt")
        eng = nc.sync if f % 2 == 0 else nc.gpsimd
        eng.dma_start(out=dwt, in_=g.dw_d[f][:, jc])
        for t2 in range(2):
            nc.tensor.matmul(
                dn[t2], g.hid_tiles[f][:, t2 * P:(t2 + 1) * P], dwt,
                start=(f == 0), stop=(f == NF - 1),
            )

    def finish(j):
        jc = slice(j * 512, (j + 1) * 512)
        dn = state["dn"]
        for t2 in range(2):
            ti = q * 2 + t2
            x2s = outp.tile([P, 512], bf16, name="x2s")
            nc.gpsimd.dma_start(out=x2s, in_=g.x2_d[ti * P:(ti + 1) * P, jc])
            ot = outp.tile([P, 512], f32, name="ot")
            if g.has_db:
                nc.vector.tensor_add(out=ot, in0=dn[t2], in1=g.db_bc[:, jc])
                nc.vector.tensor_add(out=ot, in0=ot, in1=x2s)
            else:
                nc.vector.tensor_add(out=ot, in0=dn[t2], in1=x2s)
            nc.sync.dma_start(out=g.out_d[ti * P:(ti + 1) * P, jc], in_=ot)

    steps = [(lambda f=f: up_step(f)) for f in range(NF)]
    for j in range(2):
        steps += [(lambda j=j, f=f: down_step(j, f)) for f in range(NF)]
        steps.append(lambda j=j: finish(j))
    return steps


def _emit_mlp_quarter_wide(g, q, hidp, dwps, outp, upps, dnps):
    """Tail variant: f-outer down with full-width dw loads and 2x[P,E] dn."""
    nc = g.nc
    qcols = slice(q * TQQ, (q + 1) * TQQ)
    dn = [dnps.tile([P, E], f32, name=f"dnw{j}") for j in range(2)]
    for f in range(NF):
        pu = upps.tile([P, TQQ], f32, name="py")
        for c in range(NE):
            nc.tensor.matmul(
                pu, g.uw_sb[c][:, f * P:(f + 1) * P], g.h2T[c][:, qcols],
                start=(c == 0), stop=(c == NE - 1),
            )
        hid = g.hid_tiles[f]
        nc.scalar.activation(out=hid, in_=pu, func=AF.Relu,
                             bias=g.ub_sb[:, f:f + 1])
        dwt = dwps.tile([P, E], bf16, name="dwf")
        nc.sync.dma_start(out=dwt, in_=g.dw_d[f])
        for t2 in range(2):
            for j in range(2):
                nc.tensor.matmul(
                    dn[t2][:, j * 512:(j + 1) * 512],
                    hid[:, t2 * P:(t2 + 1) * P],
                    dwt[:, j * 512:(j + 1) * 512],
                    start=(f == 0), stop=(f == NF - 1),
                )
    for t2 in range(2):
        ti = q * 2 + t2
        x2s = outp.tile([P, E], bf16, name="x2w")
        nc.gpsimd.dma_start(out=x2s, in_=g.x2_d[ti * P:(ti + 1) * P, :])
        ot = outp.tile([P, E], f32, name="otw")
        if g.has_db:
            nc.vector.tensor_add(out=ot, in0=dn[t2], in1=g.db_bc)
            nc.vector.tensor_add(out=ot, in0=ot, in1=x2s)
        else:
            nc.vector.tensor_add(out=ot, in0=dn[t2], in1=x2s)
        nc.sync.dma_start(out=g.out_d[ti * P:(ti + 1) * P, :], in_=ot)





def _build(flags, reps=1):
    has_qb, has_pb, has_db = flags
    nc = bacc.Bacc("TRN2", target_bir_lowering=False, debug=False, num_devices=8)

    g = _Ctx()
    g.nc = nc
    g.has_qb, g.has_pb, g.has_db = flags
    g.xkv_d = nc.dram_tensor("xkv", [T, E], f32, kind="ExternalInput")
    g.wq_d = nc.dram_tensor("wq", [NB, NPAIR, P, 2 * P], f8, kind="ExternalInput")
    g.wk_d = nc.dram_tensor("wk", [NB, NPAIR, P, 2 * P], f8, kind="ExternalInput")
    g.wv_d = nc.dram_tensor("wv", [NB, P, 2 * E], f8, kind="ExternalInput")
    g.vrow_d = nc.dram_tensor("vrow", [1, 2 * H * VP], f8, kind="ExternalInput")
    g.pw_d = nc.dram_tensor("pw", [NB, P, 2 * E], f8, kind="ExternalInput")
    g.uw_d = nc.dram_tensor("uw", [NE, P, F], bf16, kind="ExternalInput")
    g.ub_d = nc.dram_tensor("ub", [P, NF], f32, kind="ExternalInput")
    g.dw_d = nc.dram_tensor("dw", [NF, P, E], bf16, kind="ExternalInput")
    if has_qb:
        g.qb_d = nc.dram_tensor("qb", [P, NPAIR], f32, kind="ExternalInput")
        g.kb_d = nc.dram_tensor("kb", [P, NPAIR], f32, kind="ExternalInput")
        g.vbrow_d = nc.dram_tensor("vbrow", [1, E], bf16, kind="ExternalInput")
    if has_pb:
        g.pbrow_d = nc.dram_tensor("pbrow", [1, E], f32, kind="ExternalInput")
    if has_db:
        g.dbrow_d = nc.dram_tensor("dbrow", [1, E], f32, kind="ExternalInput")
    g.x2_d = nc.dram_tensor("x2s", [TQ, E], bf16, kind="Internal")
    g.out_d = nc.dram_tensor("out", [TQ, E], f32, kind="ExternalOutput")

    with tile.TileContext(nc) as tc:
        with (
            tc.tile_pool(name="consts", bufs=1) as consts,
            tc.tile_pool(name="stat", bufs=4) as stat,
            tc.tile_pool(name="catp", bufs=1) as catp,
            tc.tile_pool(name="x2p", bufs=1) as x2p,
            tc.tile_pool(name="h2Tp", bufs=1) as h2Tp,
        ):
            g.consts, g.stat = consts, stat
            _emit_consts(g)
            for _rep in range(reps):
                _emit_all(g, tc, catp, x2p, h2Tp)

    nc.finalize()
    return nc


def _emit_all(g, tc, catp, x2p, h2Tp):
    g.catT2 = [catp.tile([P, 2 * TQ], f8, name=f"catT{u}") for u in range(NB)]
    g.h2T = [h2Tp.tile([P, TQ], bf16, name=f"h2T{c}") for c in range(NE)]

    g.pwp = tc.alloc_tile_pool(name="pwp", bufs=1)
    g.uwp = tc.alloc_tile_pool(name="uwp", bufs=1)
    hidp = tc.alloc_tile_pool(name="hidp", bufs=1)
    g.hid_tiles = [hidp.tile([P, TQQ], bf16, name=f"hid{f}")
                   for f in range(NF)]

    with tc.tile_pool(name="vaug", bufs=1) as vap:
        g.va2 = [vap.tile([P, 2 * H * VP], f8, name=f"va{u}")
                 for u in range(NSU)]
        qktp = tc.alloc_tile_pool(name="qktp", bufs=1)
        qts = [qktp.tile([P, TQ], f8, name=f"qt{p}") for p in range(NPAIR)]
        kts = [qktp.tile([P, T], f8, name=f"kt{p}") for p in range(NPAIR)]

        with tc.tile_pool(name="hTp", bufs=1) as hTp:
            g.hT2 = [hTp.tile([P, 2 * T], f8, name=f"hT{b}") for b in range(NB)]
            with (
                tc.tile_pool(name="hp", bufs=4) as hp,
                tc.tile_pool(name="xk", bufs=3) as xkp,
                tc.tile_pool(name="tps", bufs=4, space="PSUM") as tps,
                tc.tile_pool(name="wvp", bufs=1) as wvp,
                tc.tile_pool(name="vps", bufs=2, space="PSUM") as vps,
            ):
                g.hp = hp
                _emit_ln1_v(g, xkp, tps, wvp, vps)

            g.pw_sb = []
            for u in range(NB):
                w = g.pwp.tile([P, 2 * E], f8, name=f"pw{u}")
                g.nc.gpsimd.dma_start(out=w, in_=g.pw_d[u])
                g.pw_sb.append(w)

            # th=0 attention; Q/K for pair p+1 emitted as PE filler under
            # pair p's exp stream (pair 0's Q/K runs upfront)
            with (
                tc.tile_pool(name="ptp0", bufs=3) as ptp0,
                tc.tile_pool(name="smp0", bufs=1) as smp0,
                tc.tile_pool(name="wqk", bufs=6) as wqkp,
                tc.tile_pool(name="atps", bufs=2, space="PSUM") as atps,
                tc.tile_pool(name="scps", bufs=4, space="PSUM") as scps,
            ):
                for st in _qk_steps(g, 0, qts[0], kts[0], wqkp, scps):
                    st()
                qk_queue = []
                qi = {"i": 0}

                def qk_filler(n=1):
                    for _ in range(n):
                        if qi["i"] < len(qk_queue):
                            qk_queue[qi["i"]]()
                            qi["i"] += 1

                for p in range(NPAIR):
                    if p + 1 < NPAIR:
                        qk_queue.extend(
                            _qk_steps(g, p + 1, qts[p + 1], kts[p + 1],
                                      wqkp, scps))
                    _emit_attn_pair(g, p, 0, qts[p], kts[p], ptp0, smp0,
                                    scps, atps, filler=qk_filler)
                qk_filler(len(qk_queue))

        # hT2 freed; proj th0, then th1 attention with MLP q0/q1 as filler
        with (
            tc.tile_pool(name="ptp", bufs=4) as ptp,
            tc.tile_pool(name="smp", bufs=1) as smp,
            tc.tile_pool(name="xq2", bufs=2) as xq2p,
            tc.tile_pool(name="h2p", bufs=2) as h2p,
            tc.tile_pool(name="dwps", bufs=4) as dwps,
            tc.tile_pool(name="outp", bufs=2) as outp,
        ):
            g.uw_sb = []
            for c in range(NE):
                w = g.uwp.tile([P, F], bf16, name=f"uw{c}")
                g.nc.gpsimd.dma_start(out=w, in_=g.uw_d[c])
                g.uw_sb.append(w)
            with (
                tc.tile_pool(name="pps", bufs=2, space="PSUM") as pps,
                tc.tile_pool(name="t2ps", bufs=2, space="PSUM") as t2ps,
            ):
                for st in _proj_steps(g, 0, xq2p, h2p, pps, t2ps):
                    st()
            with (
                tc.tile_pool(name="atps2", bufs=1, space="PSUM") as atps2,
                tc.tile_pool(name="scps2", bufs=2, space="PSUM") as scps2,
                tc.tile_pool(name="pup", bufs=2, space="PSUM") as pup,
                tc.tile_pool(name="dnp", bufs=1, space="PSUM") as dnp,
            ):
                steps = (_mlp_steps(g, 0, pup, dnp, dwps, outp)
                         + _mlp_steps(g, 1, pup, dnp, dwps, outp))
                si = {"i": 0}

                def filler(n=3):
                    for _ in range(n):
                        if si["i"] < len(steps):
                            steps[si["i"]]()
                            si["i"] += 1

                for p in range(NPAIR):
                    _emit_attn_pair(g, p, 1, qts[p], kts[p], ptp, smp,
                                    scps2, atps2, filler=filler)
                filler(len(steps))  # flush leftovers
        qktp.release()

    # proj-th1 interleaved with the MLP tail: quarter q2 needs only the
    # first two proj tiles (ts4/ts5), so it runs on the PE while ts6/ts7's
    # LN2 chains occupy DVE/Act; q3 follows ts7.
    with (
        tc.tile_pool(name="xq2b", bufs=2) as xq2b,
        tc.tile_pool(name="h2pb", bufs=2) as h2pb,
        tc.tile_pool(name="dwpsT", bufs=4) as dwpsT,
        tc.tile_pool(name="outpT", bufs=2) as outpT,
        tc.tile_pool(name="ppsB", bufs=2, space="PSUM") as ppsB,
        tc.tile_pool(name="dnpsT", bufs=1, space="PSUM") as dnpsT,
    ):
        psteps = _proj_steps(g, 1, xq2b, h2pb, ppsB, ppsB)
        psteps[0]()
        psteps[1]()
        _emit_mlp_quarter_wide(g, 2, hidp, dwpsT, outpT, ppsB, dnpsT)
        psteps[2]()
        psteps[3]()
        _emit_mlp_quarter_wide(g, 3, hidp, dwpsT, outpT, ppsB, dnpsT)
    hidp.release()
    g.uwp.release()
    g.pwp.release()


def _get_nc(flags, reps=1):
    key = (flags, reps)
    if key not in _BUILD_CACHE:
        _BUILD_CACHE[key] = _build(flags, reps)
    return _BUILD_CACHE[key]


def _q8(x):
    return np.clip(np.asarray(x, np.float32), -240.0, 240.0).astype(
        ml_dtypes.float8_e4m3)


def _prep(x, Wq, Wk, Wv, proj_w, proj_b, ln1_g, ln1_b, ln2_g, ln2_b,
          up_w, up_b, down_w, down_b):
    """Host-side shard + weight fold/cast/layout. Returns (flags, in_maps)."""
    bfl = ml_dtypes.bfloat16
    x = np.ascontiguousarray(np.asarray(x, dtype=np.float32))
    Wq = np.asarray(Wq, np.float32)
    Wk = np.asarray(Wk, np.float32)
    Wv = np.asarray(Wv, np.float32)
    g1 = np.asarray(ln1_g, np.float32)
    b1 = np.asarray(ln1_b, np.float32)
    g2 = np.asarray(ln2_g, np.float32)
    b2 = np.asarray(ln2_b, np.float32)
    proj_w = np.asarray(proj_w, np.float32)
    up_w = np.asarray(up_w, np.float32)
    down_w = np.asarray(down_w, np.float32)

    # [H, E, D] -> [E, H*D]; fold attention scale into Q, LN1 gain into all
    wq_all = (Wq * (D ** -0.5)).transpose(1, 0, 2).reshape(E, E)
    wk_all = Wk.transpose(1, 0, 2).reshape(E, E)
    wv_all = Wv.transpose(1, 0, 2).reshape(E, E)
    qb_vec = b1 @ wq_all
    kb_vec = b1 @ wk_all
    vb_vec = b1 @ wv_all
    wq_f = g1[:, None] * wq_all
    wk_f = g1[:, None] * wk_all
    wv_f = g1[:, None] * wv_all

    def _pair_chunks_qk(w):  # [E, E] -> [NB, NPAIR, P, 2P], DR pair layout
        return np.ascontiguousarray(
            _q8(w.reshape(NB, 2, P, NPAIR, P).transpose(0, 3, 2, 1, 4)
                .reshape(NB, NPAIR, P, 2 * P)))

    def _pair_rows(w, ncols):  # [E_in, ncols] -> [E_in/256, P, 2*ncols]
        nb = w.shape[0] // 256
        return np.ascontiguousarray(
            _q8(w.reshape(nb, 2, P, ncols).transpose(0, 2, 1, 3)
                .reshape(nb, P, 2 * ncols)))

    vrow = np.zeros((1, 2 * H * VP), np.float32)
    vrow.reshape(2, H, VP)[:, :, D] = 1.0

    uw_f = g2[:, None] * up_w
    ub_f = np.asarray(up_b, np.float32) + b2 @ up_w

    has_qb = bool(np.any(b1 != 0))
    has_pb = bool(np.any(np.asarray(proj_b) != 0))
    has_db = bool(np.any(np.asarray(down_b) != 0))
    flags = (has_qb, has_pb, has_db)

    shared = {
        "wq": _pair_chunks_qk(wq_f),
        "wk": _pair_chunks_qk(wk_f),
        "wv": _pair_rows(wv_f, E),
        "vrow": _q8(vrow),
        "pw": _pair_rows(proj_w, E),
        "uw": np.ascontiguousarray(uw_f.reshape(NE, P, F).astype(bfl)),
        "ub": np.ascontiguousarray(ub_f.reshape(NF, P).T.astype(np.float32)),
        "dw": np.ascontiguousarray(down_w.reshape(NF, P, E).astype(bfl)),
    }
    if has_qb:
        shared["qb"] = np.ascontiguousarray(
            qb_vec.reshape(NPAIR, P).T.astype(np.float32))
        shared["kb"] = np.ascontiguousarray(
            kb_vec.reshape(NPAIR, P).T.astype(np.float32))
        shared["vbrow"] = vb_vec.reshape(1, E).astype(bfl)
    if has_pb:
        shared["pbrow"] = np.asarray(proj_b, np.float32).reshape(1, E)
    if has_db:
        shared["dbrow"] = np.asarray(down_b, np.float32).reshape(1, E)

    in_maps = []
    for c in range(8):
        b, half = c // 2, c % 2
        xb = x[b]
        if half == 1:
            xb = np.concatenate([xb[TQ:], xb[:TQ]], axis=0)
        in_maps.append({"xkv": np.ascontiguousarray(xb), **shared})
    return flags, in_maps


def kernel(**inputs) -> np.ndarray:
    flags, in_maps = _prep(**inputs)
    nc = _get_nc(flags)
    res = run_bass_kernel_spmd(nc, in_maps, core_ids=list(range(8)))
    out = np.empty((B, T, E), np.float32)
    for c in range(8):
        b, half = c // 2, c % 2
        out[b, half * TQ:(half + 1) * TQ, :] = res.results[c]["out"]
    return out
